# revision 1
# baseline (speedup 1.0000x reference)
"""Trainium2 Bass kernel for nn_Attention_57166014709861.

8-batch image attention (B=8, C=384, h=8, d=48, HW=1024), data-parallel:
one batch image per NeuronCore, weights broadcast, host-side gather.
~110us HW exec on a cold TRN2 chip (baseline of this session: 138us; the
chip power-throttles when hot, inflating any measurement by up to +30%
-- see throttle_* fields in the NTFF summary before comparing numbers).

Per-core pipeline (all matmuls bf16; inputs pre-laid-out host-side to
[partition, chunk, free] and shipped as bf16 to halve the load ramp;
x is split into one tile per ko chunk because tile-granular dependency
tracking otherwise gates the first matmul on ALL input DMAs):
  qkv:   q,k packed [d,seq] head-pair tiles (2 heads per 128 partitions at
         offsets 0/64, scale folded into wq); v computed transposed as
         vT [seq, packed-c] with a leading ones column per head so the
         softmax denominator rides the av matmul for free (psum row 0).
         psum->sbuf copies alternate DVE/ACT (GPSIMD cannot read PSUM).
  attn:  per head pair, interleaved through the y-tile loop with a
         pair-shared av psum tile (rows 0:64 / 64:128) and THREE sT psum
         slots. The exp stream is split across TWO engines (6:5): ACT runs
         native Exp; DVE computes exp via a Schraudolph-style
         tensor_scalar bit trick (uint16 bits = s*184.665 + 16249.125
         truncated == bf16(exp(s)), zero-mean +-4% error; this is the
         dominant term of the ~1e-2 rel err: softmax-argument noise does
         NOT average away, so B MUST be zero-mean-calibrated and anything
         coarser than bf16 on the q/k path blows the gate). Attention is
         PE-bound at ~2.0us per (pair, y-tile) iteration: 8 matmuls of
         512 rows at 2.4 GHz plus ~260ns of LDWEIGHTS bubbles.
  norm:  denominator rows are DMA-transposed to [128,16] via a DRAM
         bounce (direct sbuf->sbuf scatters contend with PE SBUF ports
         mid-attention), reciprocal in bf16 on DVE, broadcast back with a
         stride-0 DMA read (~8us latency, hidden), one Pool multiply per
         pair (all-SBUF, keeps DVE free for exp). Pipelined one pair
         behind; epi_a fires at the next pair's yt1, epi_b at yt3, so u2
         is ready right as the last pair's av finishes. The FINAL pair
         skips the broadcast DMA: sbuf->sbuf scatter to 128 lanes,
         reciprocal, scatter back to rows {0,32}, then K=1 ones-matmul
         broadcasts on the PE into PSUM (runs behind proj kt0-2), DVE
         multiply straight from PSUM.
  proj:  w_projT packed on K with zero pad rows; kt0-2 of all three output
         tiles run while the last pair's epilogue drains, kt3 lands last;
         bias added during the psum->sbuf copy, spread ACT/DVE/ACT, with
         the three bf16 out DMAs on three different queues.
"""

import sys

if "/opt/trn_rl_repo" not in sys.path:
    sys.path.insert(0, "/opt/trn_rl_repo")

import numpy as np

import concourse.bass as bass
import concourse.mybir as mybir
import concourse.tile as tile
from concourse import bacc
from concourse.bass_utils import run_bass_kernel_spmd

DIM = 384
HEADS = 8
DH = 48
SEQ = 1024
P = 128
NCORES = 8
VP = 64  # packed v cols per head: ones at col 0, zeros 1-15, 48 data at 16-63

F32 = mybir.dt.float32
BF16 = mybir.dt.bfloat16
U16 = mybir.dt.uint16
EXP = mybir.ActivationFunctionType.Exp
IDENT = mybir.ActivationFunctionType.Identity
ADD = mybir.AluOpType.add
MULT = mybir.AluOpType.mult

# NOTE: fp8 (e4m3) for the q/k convs or the attention matmuls FAILS the
# 2e-2 gate (measured 4.9e-2): perturbations of the softmax argument do
# NOT wash out -- the attention output is itself a p-weighted average, so
# its magnitude shrinks by the same 1/sqrt(n_eff) as the noise and the
# relative error stays ~= the per-element exp perturbation (~6% for fp8
# q,k). Keep everything on the s-path at bf16 or better.

# Schraudolph exp-as-bf16-bits: trunc(s*A + B) viewed as bf16 ~= exp(s).
# A = 128/ln2; B calibrated for ZERO-MEAN rel err (max 4.2%, rms 1.8%)
# under truncation — a nonzero mean would act as a per-y-block softmax
# temperature shift that does NOT wash out in the normalization.
A_EXP = 184.6649652337873
B_EXP = 16249.125

_NC_CACHE = {}


def _emit(tc, nc, x_d, wq_d, wk_d, wv_d, wp_d, b_d, out_d):
    with (
        tc.tile_pool(name="const", bufs=1) as constp,
        tc.tile_pool(name="weights", bufs=1) as wpool,
        tc.tile_pool(name="data", bufs=1) as data,
        tc.tile_pool(name="ptile", bufs=8) as ppool,
        tc.tile_pool(name="bcpool", bufs=3) as bcpool,
        tc.tile_pool(name="rpool", bufs=3) as rpool,
        tc.tile_pool(name="avcp", bufs=2) as avcp,
        tc.tile_pool(name="opool", bufs=2) as opool,
        tc.tile_pool(name="ps_s", bufs=3, space="PSUM") as ps_s,
        tc.tile_pool(name="ps_av", bufs=1, space="PSUM") as ps_av,
        tc.tile_pool(name="dram", bufs=3, space="DRAM") as drampool,
    ):
        # ---- loads (bf16, 3 queues). Dependency tracking is per-TILE, so
        # x is split into one tile per ko chunk — the first q matmul then
        # only waits for x[ko0] + wq (~3us) instead of every input DMA
        # (~13us). Whole-tensor DMAs keep per-partition lines >= 2KB (1KB
        # lines run at ~half DMA throughput). ----
        x_ch = [
            data.tile([P, SEQ], BF16, tag=f"x{ko}", name=f"x{ko}")
            for ko in range(3)
        ]
        wq_sb = wpool.tile([P, 3, 512], BF16, tag="wq")
        wk_sb = wpool.tile([P, 3, 512], BF16, tag="wk")
        wv_sb = wpool.tile([P, 3, HEADS * VP], BF16, tag="wv")
        wp_sb = wpool.tile([P, 4, DIM], BF16, tag="wp")
        bias_sb = constp.tile([P, 3], F32, tag="bias")

        nc.sync.dma_start(x_ch[0][:], x_d.ap()[:, 0, :])
        nc.scalar.dma_start(wq_sb[:], wq_d.ap())
        nc.gpsimd.dma_start(wk_sb[:], wk_d.ap())
        nc.sync.dma_start(x_ch[1][:], x_d.ap()[:, 1, :])
        nc.scalar.dma_start(wv_sb[:], wv_d.ap())
        nc.gpsimd.dma_start(wp_sb[:], wp_d.ap())
        nc.sync.dma_start(x_ch[2][:], x_d.ap()[:, 2, :])
        nc.scalar.dma_start(bias_sb[:], b_d.ap())

        zb_sb = constp.tile([P, 1], F32, tag="zb")
        nc.gpsimd.memset(zb_sb[:], 0.0)
        # rows 0 and 32 are used as K=1 matmul lhsT (base partition must be
        # 0/32/64), so the broadcast ones tile spans 33 partitions.
        ones_sb = constp.tile([33, 64], BF16, tag="ones")
        nc.gpsimd.memset(ones_sb[:], 1.0)

        # ---- qkv (interleave q/k per pair so attention could start early;
        # psum->sbuf copies rotate across DVE/ACT/Pool to keep DVE free) ----
        q_sb = data.tile([P, 4, SEQ], BF16, tag="q")
        k_sb = data.tile([P, 4, SEQ], BF16, tag="k")
        vT_sb = data.tile([P, 8, HEADS, VP], BF16, tag="vT")

        def eng_copy(n, dst, src):
            # GPSIMD cannot access PSUM, so rotate DVE/ACT only.
            if n % 2 == 0:
                nc.vector.tensor_copy(dst, src)
            else:
                nc.scalar.activation(dst, src, IDENT, bias=zb_sb[:])

        ncopy = 0
        for t in range(4):
            for dst, w in ((q_sb, wq_sb), (k_sb, wk_sb)):
                ps = ps_s.tile([P, SEQ], F32, tag="s", name="qk_ps")
                for j in range(2):
                    for ko in range(3):
                        nc.tensor.matmul(
                            ps[:, j * 512 : (j + 1) * 512],
                            lhsT=w[:, ko, t * 128 : (t + 1) * 128],
                            rhs=x_ch[ko][:, j * 512 : (j + 1) * 512],
                            start=(ko == 0),
                            stop=(ko == 2),
                        )
                eng_copy(ncopy, dst[:, t, :], ps[:])
                ncopy += 1

        for yt in range(8):
            ps = ps_s.tile([P, SEQ], F32, tag="s", name="v_ps")
            for ko in range(3):
                nc.tensor.matmul(
                    ps[:, 0 : HEADS * VP],
                    lhsT=x_ch[ko][:, yt * 128 : (yt + 1) * 128],
                    rhs=wv_sb[:, ko, :],
                    start=(ko == 0),
                    stop=(ko == 2),
                )
            eng_copy(
                ncopy,
                vT_sb[:, yt, :, :],
                ps[:, 0 : HEADS * VP].rearrange("p (h v) -> p h v", h=HEADS),
            )
            ncopy += 1
            # ones column (col 0) for the softmax denominator (psum row 0)
            nc.gpsimd.memset(vT_sb[:, yt, :, 0:1], 1.0)

        # ---- attention ----
        u_sb = [data.tile([P, SEQ], BF16, tag=f"u{i}", name=f"u{i}") for i in range(4)]

        def epi_a(t, av_copy, q=None):
            # av_copy rows 0/64 = softmax denominators of heads 2t/2t+1.
            # DMA-transpose both to [128, 16] via a DRAM bounce so the
            # reciprocal runs on 128 lanes.
            q = q or nc.sync
            den_dram = drampool.tile([2 * SEQ], F32, tag="den")
            q.dma_start(den_dram[0:SEQ], av_copy[0:1, :])
            q.dma_start(den_dram[SEQ : 2 * SEQ], av_copy[64:65, :])
            den_pm = rpool.tile([P, 16], F32, tag="denpm")
            q.dma_start(den_pm[:], den_dram[:].rearrange("(p f) -> p f", p=P))
            rec_pm = rpool.tile([P, 16], BF16, tag="recpm")
            with nc.allow_low_precision(reason="softmax denom reciprocal to bf16"):
                nc.vector.reciprocal(rec_pm[:], den_pm[:])
            rec_dram = drampool.tile([2 * SEQ], BF16, tag="rec")
            q.dma_start(rec_dram[:], rec_pm[:])
            return rec_dram

        def epi_b(t, av_copy, rec_dram, q=None):
            # one stride-0 DMA broadcasts both heads' 1/denom to [128, 1024]
            # (rows 0:64 <- head 2t, rows 64:128 <- head 2t+1), one multiply.
            q = q or nc.sync
            bc_sb = bcpool.tile([P, SEQ], BF16, tag="bcs")
            rec_r = rec_dram[:].rearrange("(h f) -> h f", h=2)
            q.dma_start(bc_sb[:], rec_r[:, None, :].to_broadcast([2, 64, SEQ]))
            # all-SBUF multiply -> Pool engine (keeps DVE free for exp)
            nc.gpsimd.tensor_tensor(u_sb[t][:], av_copy[:], bc_sb[:], MULT)

        # exp engine rotation: ACT (native exp) : DVE (bit trick) ~ 6:5 by
        # engine throughput (996 / 1190 ns per [128,1024] tile). GPSIMD
        # cannot read PSUM, so it can't join the exp pool.
        EPAT = "ADADADADADA"

        def do_exp(n, p_tile, sT_ps):
            if EPAT[n % 11] == "A":
                nc.scalar.activation(p_tile[:], sT_ps[:], EXP, bias=zb_sb[:])
            else:
                nc.vector.tensor_scalar(
                    p_tile[:], sT_ps[:], A_EXP, B_EXP, MULT, ADD
                )

        # Head-pair interleaved with a pair-shared av tile (rows 0:64 / 64:128)
        # and THREE sT psum slots: the PE runs a y-tile ahead of the exp
        # stream, so the exp engines never gate the PE.
        pending = []
        ntile = 0
        for t in range(4):
            av_ps = ps_av.tile([P, SEQ], F32, tag="av", name=f"av{t}")
            p_tiles = [[None] * 8 for _ in range(2)]
            p_bf = [[None] * 8 for _ in range(2)]
            for yt in range(9):
                # NOTE: this [sT both heads, then av both heads] order beats
                # the per-head [sT, av] interleave by ~14us/run at matched
                # throttle — do not "improve" it.
                for s in range(2):
                    po = s * 64
                    if yt < 8:
                        sT_ps = ps_s.tile([P, SEQ], F32, tag="s", name="sT_ps")
                        for j in range(2):
                            nc.tensor.matmul(
                                sT_ps[:, j * 512 : (j + 1) * 512],
                                lhsT=k_sb[
                                    po : po + 48, t, yt * 128 : (yt + 1) * 128
                                ],
                                rhs=q_sb[po : po + 48, t, j * 512 : (j + 1) * 512],
                                start=True,
                                stop=True,
                            )
                        if EPAT[ntile % 11] == "A":
                            pt = ppool.tile([P, SEQ], BF16, tag="p", name="p_sb")
                            p_bf[s][yt] = pt[:]
                        else:
                            pt = ppool.tile([P, SEQ], U16, tag="p", name="p_sb")
                            p_bf[s][yt] = pt[:].bitcast(BF16)
                        do_exp(ntile, pt, sT_ps)
                        p_tiles[s][yt] = pt
                        ntile += 1
                for s in range(2):
                    po = s * 64
                    h = 2 * t + s
                    if yt > 0:
                        for j in range(2):
                            nc.tensor.matmul(
                                av_ps[po : po + VP, j * 512 : (j + 1) * 512],
                                lhsT=vT_sb[:, yt - 1, h, :],
                                rhs=p_bf[s][yt - 1][:, j * 512 : (j + 1) * 512],
                                start=(yt == 1),
                                stop=(yt == 8),
                                skip_group_check=True,
                            )
                # fire the previous pair's epilogue early so its u tile is
                # ready before the proj matmuls need it (the broadcast DMA
                # alone has ~8us of descriptor-split latency)
                if yt == 1 and pending and len(pending[0]) == 2:
                    pending[0] = (*pending[0], epi_a(*pending[0]))
                if yt == 3 and pending and len(pending[0]) == 3:
                    epi_b(*pending[0])
                    pending = []
            av_copy = avcp.tile([P, SEQ], F32, tag="avc", name=f"avc{t}")
            nc.vector.tensor_copy(av_copy[:], av_ps[:])
            pending.append((t, av_copy))

        # final pair: scatter the denominator rows straight out of PSUM (in
        # parallel with the av_copy on DVE), reciprocal on 128 lanes,
        # scatter back to a [2, 1024] row pair, then a K=1 ones-matmul
        # broadcast on the PE instead of the slow stride-0 broadcast DMA
        # (~8us of descriptor-split latency).
        fin = []
        for pend in pending:
            t, av_copy = pend
            den_pm = rpool.tile([P, 16], F32, tag="denpm")
            nc.sync.dma_start(den_pm[:, 0:8], av_copy[0:1, :])
            nc.scalar.dma_start(den_pm[:, 8:16], av_copy[64:65, :])
            rec_pm = rpool.tile([P, 16], BF16, tag="recpm")
            with nc.allow_low_precision(reason="softmax denom reciprocal to bf16"):
                nc.vector.reciprocal(rec_pm[:], den_pm[:])
            rec_sb = rpool.tile([33, SEQ], BF16, tag="recsb")
            nc.sync.dma_start(rec_sb[0:1, :], rec_pm[:, 0:8])
            nc.scalar.dma_start(rec_sb[32:33, :], rec_pm[:, 8:16])
            fin.append((t, av_copy, rec_sb))

        # ---- proj ----
        # kt3 (heads 6/7) lands last; run kt0-2 of all three output tiles
        # first so the PE stays busy while the final pair's epilogue drains.
        pr_ps = []
        for ot in range(3):
            # NOTE: ot2 on ps_av is load-balancing, not leftover: putting all
            # three groups on ps_s contends with the final pair's sT tiles.
            pool = ps_s if ot < 2 else ps_av
            ps = pool.tile(
                [P, SEQ], F32, tag=("s" if ot < 2 else "av"), name=f"prps{ot}"
            )
            pr_ps.append(ps)
            for j in range(2):
                for kt in range(3):
                    nc.tensor.matmul(
                        ps[:, j * 512 : (j + 1) * 512],
                        lhsT=wp_sb[:, kt, ot * 128 : (ot + 1) * 128],
                        rhs=u_sb[kt][:, j * 512 : (j + 1) * 512],
                        start=(kt == 0),
                        stop=False,
                    )

        # final-pair broadcast + normalize: the 4 tiny K=1 matmuls run after
        # kt0-2 (the PE is in-order; kt0-2 fills the epilogue latency),
        # then u3 (DVE) lands just before kt3 needs it.
        for t, av_copy, rec_sb in fin:
            bc_ps = ps_s.tile([P, SEQ], F32, tag="s", name="bc_ps")
            for s in range(2):
                sp = 32 * s
                for j in range(2):
                    nc.tensor.matmul(
                        bc_ps[s * 64 : (s + 1) * 64, j * 512 : (j + 1) * 512],
                        lhsT=ones_sb[sp : sp + 1, :],
                        rhs=rec_sb[sp : sp + 1, j * 512 : (j + 1) * 512],
                        start=True,
                        stop=True,
                    )
            nc.vector.tensor_tensor(u_sb[t][:], av_copy[:], bc_ps[:], MULT)

        oq = [nc.sync, nc.gpsimd, nc.scalar]
        for ot in range(3):
            o_sb = opool.tile([P, SEQ], BF16, tag="o")
            for j in range(2):
                nc.tensor.matmul(
                    pr_ps[ot][:, j * 512 : (j + 1) * 512],
                    lhsT=wp_sb[:, 3, ot * 128 : (ot + 1) * 128],
                    rhs=u_sb[3][:, j * 512 : (j + 1) * 512],
                    start=False,
                    stop=True,
                )
            # bias-copy spread over ACT/DVE so the three tiles drain in
            # parallel instead of serializing on ACT
            if ot == 1:
                nc.vector.tensor_scalar(
                    o_sb[:], pr_ps[ot][:], bias_sb[:, ot : ot + 1], None, ADD
                )
            else:
                nc.scalar.activation(
                    o_sb[:],
                    pr_ps[ot][:],
                    IDENT,
                    bias=bias_sb[:, ot : ot + 1],
                )
            oq[ot].dma_start(out_d.ap()[ot * 128 : (ot + 1) * 128, :], o_sb[:])


def build_nc():
    nc = bacc.Bacc("TRN2", target_bir_lowering=False, debug=False, num_devices=NCORES)
    x_d = nc.dram_tensor("x", [P, 3, SEQ], BF16, kind="ExternalInput")
    wq_d = nc.dram_tensor("wq", [P, 3, 512], BF16, kind="ExternalInput")
    wk_d = nc.dram_tensor("wk", [P, 3, 512], BF16, kind="ExternalInput")
    wv_d = nc.dram_tensor("wv", [P, 3, HEADS * VP], BF16, kind="ExternalInput")
    wp_d = nc.dram_tensor("wp", [P, 4, DIM], BF16, kind="ExternalInput")
    b_d = nc.dram_tensor("bias", [P, 3], F32, kind="ExternalInput")
    out_d = nc.dram_tensor("out", [DIM, SEQ], BF16, kind="ExternalOutput")

    with tile.TileContext(nc) as tc:
        _emit(tc, nc, x_d, wq_d, wk_d, wv_d, wp_d, b_d, out_d)
    nc.compile()
    return nc


def pack_inputs(x, w_qkv, w_proj, b_proj):
    """Host-side weight packing. Returns per-core input maps."""
    import ml_dtypes

    x = np.asarray(x, np.float32)
    w_qkv = np.asarray(w_qkv, np.float32)
    w_proj = np.asarray(w_proj, np.float32)
    b_proj = np.asarray(b_proj, np.float32)
    scale = DH ** -0.5
    w_q, w_k, w_v = w_qkv[0:DIM], w_qkv[DIM : 2 * DIM], w_qkv[2 * DIM :]

    WQ = np.zeros((DIM, 512), np.float32)
    WK = np.zeros((DIM, 512), np.float32)
    WV = np.zeros((DIM, HEADS * VP), np.float32)
    WP = np.zeros((512, DIM), np.float32)
    for h in range(HEADS):
        col = (h // 2) * 128 + (h % 2) * 64
        WQ[:, col : col + DH] = (w_q[h * DH : (h + 1) * DH] * scale).T
        WK[:, col : col + DH] = w_k[h * DH : (h + 1) * DH].T
        WV[:, h * VP + 16 : h * VP + 16 + DH] = w_v[h * DH : (h + 1) * DH].T
        WP[col + 16 : col + 16 + DH, :] = w_proj[:, h * DH : (h + 1) * DH].T
    BIAS = np.ascontiguousarray(b_proj.reshape(3, P).T)

    def pm(a, chunks):
        # [(chunks*P), f] -> [P, chunks, f] partition-major bf16 pre-layout
        return np.ascontiguousarray(
            a.reshape(chunks, P, a.shape[-1]).transpose(1, 0, 2)
        ).astype(ml_dtypes.bfloat16)

    WQp, WKp, WVp, WPp = pm(WQ, 3), pm(WK, 3), pm(WV, 3), pm(WP, 4)
    in_maps = []
    for b in range(NCORES):
        in_maps.append(
            {
                "x": pm(x[b].reshape(DIM, SEQ), 3),
                "wq": WQp,
                "wk": WKp,
                "wv": WVp,
                "wp": WPp,
                "bias": BIAS,
            }
        )
    return in_maps


def run(in_maps, trace=False):
    if "nc" not in _NC_CACHE:
        _NC_CACHE["nc"] = build_nc()
    nc = _NC_CACHE["nc"]
    res = run_bass_kernel_spmd(
        nc, in_maps, core_ids=list(range(NCORES)), trace=trace
    )
    out = np.stack(
        [res.results[i]["out"].astype(np.float32) for i in range(NCORES)]
    )
    return out.reshape(NCORES, DIM, 32, 32), res


def kernel(x, w_qkv, w_proj, b_proj):
    out, _ = run(pack_inputs(x, w_qkv, w_proj, b_proj))
    return out



# revision 9
# speedup vs baseline: 1.1118x; 1.1118x over previous
"""Trainium2 Bass kernel for nn_Attention_57166014709861.

8-batch image attention (B=8, C=384, h=8, d=48, HW=1024), data-parallel:
one batch image per NeuronCore, weights broadcast, host-side gather.

v2 redesign (from v1 ~119-129us):
  * PSUM in single-bank [128,512] halves (6 rotating slots) instead of
    3x [128,1024]: doubles the PE->exp buffering granularity so the PE
    and the exp engines decouple, and lets paired matmuls run
    CONCURRENTLY on disjoint PE sub-arrays (row groups 0/64 for the
    K=48 sT matmuls, col groups 0/64 for the M=64 av matmuls) --
    tile_position auto-derives from base partitions.
  * Static per-head exp engine split: even heads (s=0, partitions 0:48)
    use ACT native Exp; odd heads (s=1, partitions 64:113) use DVE.
    For DVE heads the Schraudolph affine map x = A*s + B is folded INTO
    the sT matmul: A rides the host-packed w_q scale, B rides an extra
    contraction row (q row 112 = 1, k row 112 = B), so DVE runs a pure
    copy-convert f32->u16 (trunc(x) viewed as bf16 bits == bf16(exp(s)),
    zero-mean +-4% err; see v1 notes below -- softmax-argument noise
    does NOT average away, so B must stay zero-mean-calibrated and
    anything coarser than bf16 on the q/k path blows the 2e-2 gate).
  * Output tail: opool bufs=3 so the third bias-copy does not wait for
    the first out-DMA to drain its staging buffer (v1 lost ~2.5us).
  * Input DMA: wq/wk split into ko0 + ko12 chunks so the first q/k
    matmuls start ~2us earlier; x on sync queue, wq on scalar, wk on
    gpsimd; wv/wp trail behind (v needs all of x anyway).

Per-core pipeline:
  qkv:   q,k packed [d,seq] head-pair tiles (2 heads per 128 partitions
         at offsets 0/64, scale -- and A for odd heads -- folded into
         wq); v computed transposed as vT [seq, packed-c] with a leading
         ones column per head so the softmax denominator rides the av
         matmul for free (psum row 0/64). psum->sbuf copies alternate
         DVE/ACT per [128,512] half (GPSIMD cannot read PSUM).
  attn:  per head pair t, 8 y-tiles; per yt: sT halves issued
         s0j0,s1j0,s0j1,s1j1 (s-pairs concurrent on row groups), exp'd
         ACT(s0)/DVE(s1); av one yt behind with the pair-shared av psum
         (rows 0:64 / 64:128, s-pairs concurrent on col groups).
  norm:  denominator rows DMA-transposed to [128,16] via a DRAM bounce,
         reciprocal in bf16 on DVE, broadcast back with a stride-0 DMA
         read (~8us latency, hidden), one Pool multiply per pair.
         Pipelined one pair behind; epi_a fires at the next pair's yt1,
         epi_b at yt3. The FINAL pair skips the broadcast DMA:
         sbuf->sbuf scatter, reciprocal, scatter back to rows {0,32},
         K=1 ones-matmul broadcast on the PE, DVE multiply from PSUM.
  proj:  w_projT packed on K with zero pad rows; kt0-2 of all three
         output tiles run while the last pair's epilogue drains, kt3
         lands last; bias added during the psum->sbuf copy, spread
         ACT/DVE/ACT, three bf16 out DMAs on three different queues.
"""

import sys

if "/opt/trn_rl_repo" not in sys.path:
    sys.path.insert(0, "/opt/trn_rl_repo")

import numpy as np

import concourse.bass as bass
import concourse.mybir as mybir
import concourse.tile as tile
from concourse import bacc
from concourse.bass_utils import run_bass_kernel_spmd

DIM = 384
HEADS = 8
DH = 48
SEQ = 1024
P = 128
NCORES = 8
VP = 64  # packed v cols per head: ones at col 0, zeros 1-15, 48 data at 16-63

F32 = mybir.dt.float32
BF16 = mybir.dt.bfloat16
U16 = mybir.dt.uint16
EXP = mybir.ActivationFunctionType.Exp
IDENT = mybir.ActivationFunctionType.Identity
ADD = mybir.AluOpType.add
MULT = mybir.AluOpType.mult

# NOTE: fp8 (e4m3) anywhere on the matmul paths FAILS the 2e-2 gate
# (measured 4.9e-2 for q/k): perturbations do NOT wash out -- outputs are
# p-weighted averages whose magnitude shrinks by the same 1/sqrt(n_eff)
# as the noise. Keep everything at bf16 or better.

# Schraudolph exp-as-bf16-bits: trunc(s*A + B) viewed as bf16 ~= exp(s).
# A = 128/ln2; B calibrated for ZERO-MEAN rel err (max 4.2%, rms 1.8%)
# under truncation.
A_EXP = 184.6649652337873
B_EXP = 16249.125

_NC_CACHE = {}


def _emit(tc, nc, x_d, wq0_d, wq12_d, wk0_d, wk12_d, wv_d, wp_d, b_d, brow_d, out_d):
    with (
        tc.tile_pool(name="const", bufs=1) as constp,
        tc.tile_pool(name="weights", bufs=1) as wpool,
        tc.tile_pool(name="data", bufs=1) as data,
        tc.tile_pool(name="ptile", bufs=12) as ppool,
        tc.tile_pool(name="bcpool", bufs=3) as bcpool,
        tc.tile_pool(name="rpool", bufs=3) as rpool,
        tc.tile_pool(name="avcp", bufs=2) as avcp,
        tc.tile_pool(name="opool", bufs=3) as opool,
        tc.tile_pool(name="ps_h", bufs=6, space="PSUM") as ps_h,
        tc.tile_pool(name="ps_av", bufs=1, space="PSUM") as ps_av,
        tc.tile_pool(name="dram", bufs=3, space="DRAM") as drampool,
    ):
        # ---- loads (bf16, 3 queues). Dependency tracking is per-TILE:
        # x split per ko chunk, wq/wk split ko0 vs ko12 so the first q/k
        # matmuls only wait for x0+wq0 (~10.9us) instead of everything.
        x_ch = [
            data.tile([P, SEQ], BF16, tag=f"x{ko}", name=f"x{ko}")
            for ko in range(3)
        ]
        wq0_sb = wpool.tile([P, 512], BF16, tag="wq0")
        wq12_sb = wpool.tile([P, 2, 512], BF16, tag="wq12")
        wk0_sb = wpool.tile([P, 512], BF16, tag="wk0")
        wk12_sb = wpool.tile([P, 2, 512], BF16, tag="wk12")
        wv_sb = wpool.tile([P, 3, HEADS * VP], BF16, tag="wv")
        wp_sb = wpool.tile([P, 4, DIM], BF16, tag="wp")
        bias_sb = constp.tile([P, 3], F32, tag="bias")

        nc.sync.dma_start(x_ch[0][:], x_d.ap()[:, 0, :])
        nc.scalar.dma_start(wq0_sb[:], wq0_d.ap())
        nc.gpsimd.dma_start(wk0_sb[:], wk0_d.ap())
        nc.sync.dma_start(x_ch[1][:], x_d.ap()[:, 1, :])
        nc.scalar.dma_start(wq12_sb[:], wq12_d.ap())
        nc.gpsimd.dma_start(wk12_sb[:], wk12_d.ap())
        nc.sync.dma_start(x_ch[2][:], x_d.ap()[:, 2, :])
        nc.scalar.dma_start(wv_sb[:], wv_d.ap())
        nc.gpsimd.dma_start(wp_sb[:], wp_d.ap())
        nc.scalar.dma_start(bias_sb[:], b_d.ap())

        def wq_ko(ko):
            return wq0_sb[:] if ko == 0 else wq12_sb[:, ko - 1, :]

        def wk_ko(ko):
            return wk0_sb[:] if ko == 0 else wk12_sb[:, ko - 1, :]

        zb_sb = constp.tile([P, 1], F32, tag="zb")
        nc.gpsimd.memset(zb_sb[:], 0.0)
        # rows 0 and 32 are used as K=1 matmul lhsT (base partition must be
        # 0/32/64), so the broadcast ones tile spans 33 partitions.
        ones_sb = constp.tile([33, 64], BF16, tag="ones")
        nc.gpsimd.memset(ones_sb[:], 1.0)

        # ---- qkv: per (t, q/k) two [128,512] psum halves, each a 3-ko
        # accumulation; psum->sbuf copies alternate DVE/ACT ----
        q_sb = data.tile([P, 4, SEQ], BF16, tag="q")
        k_sb = data.tile([P, 4, SEQ], BF16, tag="k")
        vT_sb = data.tile([P, 8, HEADS, VP], BF16, tag="vT")

        def eng_copy(n, dst, src):
            # GPSIMD cannot access PSUM, so rotate DVE/ACT only.
            if n % 2 == 0:
                nc.vector.tensor_copy(dst, src)
            else:
                nc.scalar.activation(dst, src, IDENT, bias=zb_sb[:])

        ncopy = 0
        for t in range(4):
            for dst, wf in ((q_sb, wq_ko), (k_sb, wk_ko)):
                for j in range(2):
                    ps = ps_h.tile([P, 512], F32, tag="h", name="qk_ps")
                    for ko in range(3):
                        nc.tensor.matmul(
                            ps[:],
                            lhsT=wf(ko)[:, t * 128 : (t + 1) * 128],
                            rhs=x_ch[ko][:, j * 512 : (j + 1) * 512],
                            start=(ko == 0),
                            stop=(ko == 2),
                        )
                    eng_copy(ncopy, dst[:, t, j * 512 : (j + 1) * 512], ps[:])
                    ncopy += 1
        # Schraudolph affine rows for the odd heads: q row 112 = 1, k row
        # 112 = B -> sT(s=1) = A*s + B. Engines cannot address base
        # partition 112, DMA can (brow is a tiny host-prepared constant;
        # bf16 rounds B to 16256 -- an integer shift of B is an exact
        # constant factor on p that cancels in the normalization).
        nc.gpsimd.dma_start(q_sb[112:113, :, :], brow_d.ap()[0:1, :, :])
        nc.gpsimd.dma_start(k_sb[112:113, :, :], brow_d.ap()[1:2, :, :])

        for yt in range(8):
            ps = ps_h.tile([P, 512], F32, tag="h", name="v_ps")
            for ko in range(3):
                nc.tensor.matmul(
                    ps[:],
                    lhsT=x_ch[ko][:, yt * 128 : (yt + 1) * 128],
                    rhs=wv_sb[:, ko, :],
                    start=(ko == 0),
                    stop=(ko == 2),
                )
            eng_copy(
                ncopy,
                vT_sb[:, yt, :, :],
                ps[:].rearrange("p (h v) -> p h v", h=HEADS),
            )
            ncopy += 1
            # ones column (col 0) for the softmax denominator (psum row 0/64)
            nc.gpsimd.memset(vT_sb[:, yt, :, 0:1], 1.0)

        # ---- attention ----
        u_sb = [data.tile([P, SEQ], BF16, tag=f"u{i}", name=f"u{i}") for i in range(4)]

        def epi_a(t, av_copy, q=None):
            # av_copy rows 0/64 = softmax denominators of heads 2t/2t+1.
            # DMA-transpose both to [128, 16] via a DRAM bounce so the
            # reciprocal runs on 128 lanes.
            q = q or nc.sync
            den_dram = drampool.tile([2 * SEQ], F32, tag="den")
            q.dma_start(den_dram[0:SEQ], av_copy[0:1, :])
            q.dma_start(den_dram[SEQ : 2 * SEQ], av_copy[64:65, :])
            den_pm = rpool.tile([P, 16], F32, tag="denpm")
            q.dma_start(den_pm[:], den_dram[:].rearrange("(p f) -> p f", p=P))
            rec_pm = rpool.tile([P, 16], BF16, tag="recpm")
            with nc.allow_low_precision(reason="softmax denom reciprocal to bf16"):
                nc.vector.reciprocal(rec_pm[:], den_pm[:])
            rec_dram = drampool.tile([2 * SEQ], BF16, tag="rec")
            q.dma_start(rec_dram[:], rec_pm[:])
            return rec_dram

        def epi_b(t, av_copy, rec_dram, q=None):
            # one stride-0 DMA broadcasts both heads' 1/denom to [128, 1024]
            # (rows 0:64 <- head 2t, rows 64:128 <- head 2t+1), one multiply.
            q = q or nc.sync
            bc_sb = bcpool.tile([P, SEQ], BF16, tag="bcs")
            rec_r = rec_dram[:].rearrange("(h f) -> h f", h=2)
            q.dma_start(bc_sb[:], rec_r[:, None, :].to_broadcast([2, 64, SEQ]))
            # all-SBUF multiply -> Pool engine (keeps DVE free for exp)
            nc.gpsimd.tensor_tensor(u_sb[t][:], av_copy[:], bc_sb[:], MULT)

        # Head-pair interleaved, av one yt behind. sT halves issue
        # s0j0,s1j0 (concurrent row groups 0/64), s0j1,s1j1; av halves
        # issue s0,s1 per j (concurrent col groups 0/64). exp is
        # statically split: s0 -> ACT native Exp, s1 -> DVE copy-convert
        # (affine map folded into the matmul).
        pending = []
        for t in range(4):
            av_ps = ps_av.tile([P, SEQ], F32, tag="av", name=f"av{t}")
            p_half = [[[None] * 2 for _ in range(8)] for _ in range(2)]
            for yt in range(9):
                if yt < 8:
                    for j in range(2):
                        for s in range(2):
                            po, ke = (0, 48) if s == 0 else (64, 49)
                            hp = ps_h.tile([P, 512], F32, tag="h", name="sT_h")
                            nc.tensor.matmul(
                                hp[:],
                                lhsT=k_sb[
                                    po : po + ke, t, yt * 128 : (yt + 1) * 128
                                ],
                                rhs=q_sb[po : po + ke, t, j * 512 : (j + 1) * 512],
                                start=True,
                                stop=True,
                            )
                            if s == 0:
                                pt = ppool.tile([P, 512], BF16, tag="p", name="p_a")
                                nc.scalar.activation(
                                    pt[:], hp[:], EXP, bias=zb_sb[:]
                                )
                                p_half[s][yt][j] = pt[:]
                            else:
                                pt = ppool.tile([P, 512], U16, tag="p", name="p_d")
                                nc.vector.tensor_copy(pt[:], hp[:])
                                p_half[s][yt][j] = pt[:].bitcast(BF16)
                if yt > 0:
                    for j in range(2):
                        for s in range(2):
                            po = s * 64
                            h = 2 * t + s
                            nc.tensor.matmul(
                                av_ps[po : po + VP, j * 512 : (j + 1) * 512],
                                lhsT=vT_sb[:, yt - 1, h, :],
                                rhs=p_half[s][yt - 1][j],
                                start=(yt == 1),
                                stop=(yt == 8),
                                skip_group_check=True,
                            )
                # fire the previous pair's epilogue early so its u tile is
                # ready before the proj matmuls need it (the broadcast DMA
                # alone has ~8us of descriptor-split latency)
                if yt == 1 and pending and len(pending[0]) == 2:
                    pending[0] = (*pending[0], epi_a(*pending[0]))
                if yt == 3 and pending and len(pending[0]) == 3:
                    epi_b(*pending[0])
                    pending = []
            av_copy = avcp.tile([P, SEQ], F32, tag="avc", name=f"avc{t}")
            nc.vector.tensor_copy(av_copy[:, 0:512], av_ps[:, 0:512])
            nc.scalar.activation(
                av_copy[:, 512:SEQ], av_ps[:, 512:SEQ], IDENT, bias=zb_sb[:]
            )
            pending.append((t, av_copy))

        # final pair: scatter the denominator rows straight out of the
        # av copy, reciprocal on 128 lanes, scatter back to a [2, 1024]
        # row pair, then a K=1 ones-matmul broadcast on the PE instead of
        # the slow stride-0 broadcast DMA (~8us of descriptor latency).
        fin = []
        for pend in pending:
            t, av_copy = pend
            den_pm = rpool.tile([P, 16], F32, tag="denpm")
            nc.sync.dma_start(den_pm[:, 0:8], av_copy[0:1, :])
            nc.scalar.dma_start(den_pm[:, 8:16], av_copy[64:65, :])
            rec_pm = rpool.tile([P, 16], BF16, tag="recpm")
            with nc.allow_low_precision(reason="softmax denom reciprocal to bf16"):
                nc.vector.reciprocal(rec_pm[:], den_pm[:])
            rec_sb = rpool.tile([33, SEQ], BF16, tag="recsb")
            nc.sync.dma_start(rec_sb[0:1, :], rec_pm[:, 0:8])
            nc.scalar.dma_start(rec_sb[32:33, :], rec_pm[:, 8:16])
            fin.append((t, av_copy, rec_sb))

        # ---- proj ----
        # kt3 (heads 6/7) lands last; run kt0-2 of all three output tiles
        # first so the PE stays busy while the final pair's epilogue drains.
        pr_ps = []
        for ot in range(3):
            halves = []
            for j in range(2):
                ps = ps_h.tile([P, 512], F32, tag="h", name=f"prps{ot}{j}")
                halves.append(ps)
                for kt in range(3):
                    nc.tensor.matmul(
                        ps[:],
                        lhsT=wp_sb[:, kt, ot * 128 : (ot + 1) * 128],
                        rhs=u_sb[kt][:, j * 512 : (j + 1) * 512],
                        start=(kt == 0),
                        stop=False,
                    )
            pr_ps.append(halves)

        # final-pair broadcast + normalize: the 4 tiny K=1 matmuls run after
        # kt0-2 (the PE is in-order; kt0-2 fills the epilogue latency),
        # then u3 (DVE) lands just before kt3 needs it.
        for t, av_copy, rec_sb in fin:
            bc_ps = ps_av.tile([P, SEQ], F32, tag="av", name="bc_ps")
            for s in range(2):
                sp = 32 * s
                for j in range(2):
                    nc.tensor.matmul(
                        bc_ps[s * 64 : (s + 1) * 64, j * 512 : (j + 1) * 512],
                        lhsT=ones_sb[sp : sp + 1, :],
                        rhs=rec_sb[sp : sp + 1, j * 512 : (j + 1) * 512],
                        start=True,
                        stop=True,
                    )
            nc.vector.tensor_tensor(u_sb[t][:], av_copy[:], bc_ps[:], MULT)

        oq = [nc.sync, nc.gpsimd, nc.scalar]
        for ot in range(3):
            o_sb = opool.tile([P, SEQ], BF16, tag="o")
            for j in range(2):
                nc.tensor.matmul(
                    pr_ps[ot][j][:],
                    lhsT=wp_sb[:, 3, ot * 128 : (ot + 1) * 128],
                    rhs=u_sb[3][:, j * 512 : (j + 1) * 512],
                    start=False,
                    stop=True,
                )
            # bias-copy spread over ACT/DVE so the three tiles drain in
            # parallel instead of serializing on ACT
            for j in range(2):
                src = pr_ps[ot][j][:]
                dst = o_sb[:, j * 512 : (j + 1) * 512]
                if (ot + j) % 2 == 1:
                    nc.vector.tensor_scalar(
                        dst, src, bias_sb[:, ot : ot + 1], None, ADD
                    )
                else:
                    nc.scalar.activation(
                        dst, src, IDENT, bias=bias_sb[:, ot : ot + 1]
                    )
            oq[ot].dma_start(out_d.ap()[ot * 128 : (ot + 1) * 128, :], o_sb[:])


def build_nc():
    nc = bacc.Bacc("TRN2", target_bir_lowering=False, debug=False, num_devices=NCORES)
    x_d = nc.dram_tensor("x", [P, 3, SEQ], BF16, kind="ExternalInput")
    wq0_d = nc.dram_tensor("wq0", [P, 512], BF16, kind="ExternalInput")
    wq12_d = nc.dram_tensor("wq12", [P, 2, 512], BF16, kind="ExternalInput")
    wk0_d = nc.dram_tensor("wk0", [P, 512], BF16, kind="ExternalInput")
    wk12_d = nc.dram_tensor("wk12", [P, 2, 512], BF16, kind="ExternalInput")
    wv_d = nc.dram_tensor("wv", [P, 3, HEADS * VP], BF16, kind="ExternalInput")
    wp_d = nc.dram_tensor("wp", [P, 4, DIM], BF16, kind="ExternalInput")
    b_d = nc.dram_tensor("bias", [P, 3], F32, kind="ExternalInput")
    brow_d = nc.dram_tensor("brow", [2, 4, SEQ], BF16, kind="ExternalInput")
    out_d = nc.dram_tensor("out", [DIM, SEQ], BF16, kind="ExternalOutput")

    with tile.TileContext(nc) as tc:
        _emit(tc, nc, x_d, wq0_d, wq12_d, wk0_d, wk12_d, wv_d, wp_d, b_d, brow_d, out_d)
    nc.compile()
    return nc


def pack_inputs(x, w_qkv, w_proj, b_proj):
    """Host-side weight packing. Returns per-core input maps."""
    import ml_dtypes

    x = np.asarray(x, np.float32)
    w_qkv = np.asarray(w_qkv, np.float32)
    w_proj = np.asarray(w_proj, np.float32)
    b_proj = np.asarray(b_proj, np.float32)
    scale = DH ** -0.5
    w_q, w_k, w_v = w_qkv[0:DIM], w_qkv[DIM : 2 * DIM], w_qkv[2 * DIM :]

    WQ = np.zeros((DIM, 512), np.float32)
    WK = np.zeros((DIM, 512), np.float32)
    WV = np.zeros((DIM, HEADS * VP), np.float32)
    WP = np.zeros((512, DIM), np.float32)
    for h in range(HEADS):
        col = (h // 2) * 128 + (h % 2) * 64
        # odd heads run the DVE Schraudolph path: fold A into the scale
        qs = scale * (A_EXP if h % 2 == 1 else 1.0)
        WQ[:, col : col + DH] = (w_q[h * DH : (h + 1) * DH] * qs).T
        WK[:, col : col + DH] = w_k[h * DH : (h + 1) * DH].T
        WV[:, h * VP + 16 : h * VP + 16 + DH] = w_v[h * DH : (h + 1) * DH].T
        WP[col + 16 : col + 16 + DH, :] = w_proj[:, h * DH : (h + 1) * DH].T
    BIAS = np.ascontiguousarray(b_proj.reshape(3, P).T)

    def pm(a, chunks):
        # [(chunks*P), f] -> [P, chunks, f] partition-major bf16 pre-layout
        return np.ascontiguousarray(
            a.reshape(chunks, P, a.shape[-1]).transpose(1, 0, 2)
        ).astype(ml_dtypes.bfloat16)

    WQp, WKp, WVp, WPp = pm(WQ, 3), pm(WK, 3), pm(WV, 3), pm(WP, 4)
    WQ0 = np.ascontiguousarray(WQp[:, 0, :])
    WQ12 = np.ascontiguousarray(WQp[:, 1:3, :])
    WK0 = np.ascontiguousarray(WKp[:, 0, :])
    WK12 = np.ascontiguousarray(WKp[:, 1:3, :])
    BROW = np.empty((2, 4, SEQ), np.float32)
    BROW[0] = 1.0
    BROW[1] = B_EXP
    BROW = BROW.astype(ml_dtypes.bfloat16)
    in_maps = []
    for b in range(NCORES):
        in_maps.append(
            {
                "x": pm(x[b].reshape(DIM, SEQ), 3),
                "wq0": WQ0,
                "wq12": WQ12,
                "wk0": WK0,
                "wk12": WK12,
                "wv": WVp,
                "wp": WPp,
                "bias": BIAS,
                "brow": BROW,
            }
        )
    return in_maps


def run(in_maps, trace=False):
    if "nc" not in _NC_CACHE:
        _NC_CACHE["nc"] = build_nc()
    nc = _NC_CACHE["nc"]
    res = run_bass_kernel_spmd(
        nc, in_maps, core_ids=list(range(NCORES)), trace=trace
    )
    out = np.stack(
        [res.results[i]["out"].astype(np.float32) for i in range(NCORES)]
    )
    return out.reshape(NCORES, DIM, 32, 32), res


def kernel(x, w_qkv, w_proj, b_proj):
    out, _ = run(pack_inputs(x, w_qkv, w_proj, b_proj))
    return out


# revision 12
# speedup vs baseline: 1.1343x; 1.0203x over previous
"""Trainium2 Bass kernel for nn_Attention_57166014709861.

8-batch image attention (B=8, C=384, h=8, d=48, HW=1024), data-parallel:
one batch image per NeuronCore, weights broadcast, host-side gather.

v3 (from v2 107us, v1 119-129us): the whole kernel is ONE software
pipeline paced by the exp engines (ACT+DVE), which own the critical
path (~74us of psum->sbuf work split across two engines).

  * Skewed emission: attention blocks att(t,yt) start right after pair
    0's q/k land (~15us instead of ~33us); the remaining qkv groups
    (q1-3/k1-3, v) are sprinkled one-per-yt into the attention stream so
    the PE (which has ~50% slack at the exp-bound pace) computes them in
    the gaps and HAM stays warm.
  * PSUM in single-bank [128,512] halves (6 rotating "h" slots shared by
    qkv/v/sT/proj): paired matmuls run CONCURRENTLY on disjoint PE
    sub-arrays (row groups 0/64 for the K<=49 sT, col groups 0/64 for
    the M=64 av) -- tile_position auto-derives from base partitions.
  * Static per-head exp split: even heads (s=0, partitions 0:48) use ACT
    native Exp; odd heads (s=1, partitions 64:113) use DVE. For DVE
    heads the Schraudolph affine map x = A*s + B is folded INTO the sT
    matmul (A rides the host-packed w_q scale, B rides contraction row
    112: q row 112 = 1, k row 112 = B, delivered by a tiny dep-free DMA
    since engines cannot address base partition 112; the q/k copies only
    write partitions 0:112 so the rows survive). DVE then runs a pure
    copy-convert f32->u16: trunc(x) viewed as bf16 bits == bf16(exp(s)),
    zero-mean +-4% err. bf16 rounds B to 16256 -- an integer shift of B
    is an exact constant factor on p that cancels in the normalization.
    Softmax-argument noise does NOT average away, so B must stay
    zero-mean-calibrated and anything coarser than bf16 on the q/k path
    blows the 2e-2 gate (fp8 measured 4.9e-2).
  * Epilogue re-phased to avoid engine FIFO head-of-line blocking: the
    denominator-scatter DMAs fire at the next pair's yt1/yt0, the DVE
    reciprocal only at yt3/yt2 (its dep already landed -> no stall), the
    broadcast DMA at yt4/yt3. Pairs 0/1 normalize on GPSIMD; pair 2 on
    DVE (its broadcast lands after exp is done, and GPSIMD's queue would
    be blocked); pair 3 skips the DMA bounce entirely: 65-lane DVE
    reciprocal straight off av_copy rows 0..64, K=1 ones-matmul
    broadcast on the PE, DVE multiply from PSUM.
  * proj is kt-outer (all three output tiles' kt round together) so each
    round starts the moment its u tile lands; kt3 lands last after the
    final pair's u3.
"""

import sys

if "/opt/trn_rl_repo" not in sys.path:
    sys.path.insert(0, "/opt/trn_rl_repo")

import numpy as np

import concourse.bass as bass
import concourse.mybir as mybir
import concourse.tile as tile
from concourse import bacc
from concourse.bass_utils import run_bass_kernel_spmd

DIM = 384
HEADS = 8
DH = 48
SEQ = 1024
P = 128
NCORES = 8
VP = 64  # packed v cols per head: ones at col 0, zeros 1-15, 48 data at 16-63

F32 = mybir.dt.float32
BF16 = mybir.dt.bfloat16
U16 = mybir.dt.uint16
EXP = mybir.ActivationFunctionType.Exp
IDENT = mybir.ActivationFunctionType.Identity
ADD = mybir.AluOpType.add
MULT = mybir.AluOpType.mult

# Schraudolph exp-as-bf16-bits: trunc(s*A + B) viewed as bf16 ~= exp(s).
# A = 128/ln2; B calibrated for ZERO-MEAN rel err (max 4.2%, rms 1.8%)
# under truncation.
A_EXP = 184.6649652337873
B_EXP = 16249.125

_NC_CACHE = {}


def _emit(tc, nc, x_d, wq0_d, wq12_d, wk0_d, wk12_d, wv_d, wp_d, b_d, brow_d, out_d):
    with (
        tc.tile_pool(name="const", bufs=1) as constp,
        tc.tile_pool(name="weights", bufs=1) as wpool,
        tc.tile_pool(name="data", bufs=1) as data,
        tc.tile_pool(name="ptile", bufs=16) as ppool,
        tc.tile_pool(name="bcpool", bufs=3) as bcpool,
        tc.tile_pool(name="rpool", bufs=3) as rpool,
        tc.tile_pool(name="avcp", bufs=3) as avcp,
        tc.tile_pool(name="opool", bufs=3) as opool,
        tc.tile_pool(name="ps_h", bufs=6, space="PSUM") as ps_h,
        tc.tile_pool(name="ps_av", bufs=1, space="PSUM") as ps_av,
        tc.tile_pool(name="dram", bufs=3, space="DRAM") as drampool,
    ):
        # ---- loads (bf16, 3 queues). Dependency tracking is per-tile
        # (with subtile refinement): x per ko chunk, wq/wk split ko0 vs
        # ko12 so pair 0's q/k matmuls only wait for x+wq0/wk0. x2 rides
        # the scalar queue so all of x lands by ~13us (q0/k0 accumulate
        # all three ko chunks before the first sT can go).
        x_ch = [
            data.tile([P, SEQ], BF16, tag=f"x{ko}", name=f"x{ko}")
            for ko in range(3)
        ]
        wq0_sb = wpool.tile([P, 512], BF16, tag="wq0")
        wq12_sb = wpool.tile([P, 2, 512], BF16, tag="wq12")
        wk0_sb = wpool.tile([P, 512], BF16, tag="wk0")
        wk12_sb = wpool.tile([P, 2, 512], BF16, tag="wk12")
        wv_sb = wpool.tile([P, 3, HEADS * VP], BF16, tag="wv")
        wp_sb = wpool.tile([P, 4, DIM], BF16, tag="wp")
        bias_sb = constp.tile([P, 3], F32, tag="bias")

        q_sb = data.tile([P, 4, SEQ], BF16, tag="q")
        k_sb = data.tile([P, 4, SEQ], BF16, tag="k")
        vT_sb = data.tile([P, 8, HEADS, VP], BF16, tag="vT")

        nc.sync.dma_start(x_ch[0][:], x_d.ap()[:, 0, :])
        nc.scalar.dma_start(wq0_sb[:], wq0_d.ap())
        nc.gpsimd.dma_start(wk0_sb[:], wk0_d.ap())
        # Schraudolph affine rows (dep-free: q/k copies never write
        # partition 112+): q row 112 = 1, k row 112 = B.
        nc.gpsimd.dma_start(q_sb[112:113, :, :], brow_d.ap()[0:1, :, :])
        nc.gpsimd.dma_start(k_sb[112:113, :, :], brow_d.ap()[1:2, :, :])
        nc.sync.dma_start(x_ch[1][:], x_d.ap()[:, 1, :])
        nc.scalar.dma_start(x_ch[2][:], x_d.ap()[:, 2, :])
        nc.gpsimd.dma_start(wk12_sb[:], wk12_d.ap())
        nc.scalar.dma_start(wq12_sb[:], wq12_d.ap())
        nc.gpsimd.dma_start(wv_sb[:], wv_d.ap())
        nc.gpsimd.dma_start(wp_sb[:], wp_d.ap())
        nc.scalar.dma_start(bias_sb[:], b_d.ap())

        def wq_ko(ko):
            return wq0_sb[:] if ko == 0 else wq12_sb[:, ko - 1, :]

        def wk_ko(ko):
            return wk0_sb[:] if ko == 0 else wk12_sb[:, ko - 1, :]

        zb_sb = constp.tile([P, 1], F32, tag="zb")
        nc.gpsimd.memset(zb_sb[:], 0.0)
        # rows 0 and 64 are used as K=1 matmul lhsT for the final-pair
        # broadcast, so the ones tile spans 65 partitions.
        ones_sb = constp.tile([65, 64], BF16, tag="ones")
        nc.gpsimd.memset(ones_sb[:], 1.0)

        # ---- copy engine rotation (GPSIMD cannot read PSUM) ----
        _ncopy = [0]

        def eng_copy(dst, src):
            if _ncopy[0] % 2 == 0:
                nc.vector.tensor_copy(dst, src)
            else:
                nc.scalar.activation(
                    dst, src, IDENT, bias=zb_sb[0 : src.shape[0], :]
                )
            _ncopy[0] += 1

        # ---- qkv work groups (emitted interleaved into the attention
        # stream below). q/k copies write partitions 0:112 only, so the
        # Schraudolph rows survive. ----
        def qk_group(t, which, j):
            dst, wf = (q_sb, wq_ko) if which == "q" else (k_sb, wk_ko)
            ps = ps_h.tile([P, 512], F32, tag="h", name="qk_ps")
            for ko in range(3):
                nc.tensor.matmul(
                    ps[:],
                    lhsT=wf(ko)[:, t * 128 : (t + 1) * 128],
                    rhs=x_ch[ko][:, j * 512 : (j + 1) * 512],
                    start=(ko == 0),
                    stop=(ko == 2),
                )
            eng_copy(dst[0:112, t, j * 512 : (j + 1) * 512], ps[0:112, :])

        def v_group(yt):
            ps = ps_h.tile([P, 512], F32, tag="h", name="v_ps")
            for ko in range(3):
                nc.tensor.matmul(
                    ps[:],
                    lhsT=x_ch[ko][:, yt * 128 : (yt + 1) * 128],
                    rhs=wv_sb[:, ko, :],
                    start=(ko == 0),
                    stop=(ko == 2),
                )
            eng_copy(
                vT_sb[:, yt, :, :],
                ps[:].rearrange("p (h v) -> p h v", h=HEADS),
            )
            # ones column (col 0) for the softmax denominator (psum row 0/64)
            nc.gpsimd.memset(vT_sb[:, yt, :, 0:1], 1.0)

        # ---- attention state ----
        u_sb = [data.tile([P, SEQ], BF16, tag=f"u{i}", name=f"u{i}") for i in range(4)]
        epi = {}  # pair -> dict with av_copy / den_dram / rec_dram

        # Epilogue for pairs 0-2, phase-split so no engine FIFO-blocks on
        # a DMA that hasn't landed: den scatter DMAs early, reciprocal
        # two yts later (dep already met), broadcast next.
        def epi_den(t):
            st = epi[t]
            den_dram = drampool.tile([2 * SEQ], F32, tag="den")
            nc.sync.dma_start(den_dram[0:SEQ], st["avc"][0:1, :])
            nc.sync.dma_start(den_dram[SEQ : 2 * SEQ], st["avc"][64:65, :])
            den_pm = rpool.tile([P, 16], F32, tag="denpm")
            nc.sync.dma_start(den_pm[:], den_dram[:].rearrange("(p f) -> p f", p=P))
            st["den_pm"] = den_pm

        def epi_recip(t):
            st = epi[t]
            rec_pm = rpool.tile([P, 16], BF16, tag="recpm")
            with nc.allow_low_precision(reason="softmax denom reciprocal to bf16"):
                nc.vector.reciprocal(rec_pm[:], st["den_pm"][:])
            rec_dram = drampool.tile([2 * SEQ], BF16, tag="rec")
            nc.sync.dma_start(rec_dram[:], rec_pm[:])
            st["rec_dram"] = rec_dram

        def epi_bcast(t, mult_engine):
            st = epi[t]
            bc_sb = bcpool.tile([P, SEQ], BF16, tag="bcs")
            rec_r = st["rec_dram"][:].rearrange("(h f) -> h f", h=2)
            nc.sync.dma_start(bc_sb[:], rec_r[:, None, :].to_broadcast([2, 64, SEQ]))
            if mult_engine == "gpsimd":
                # all-SBUF multiply on Pool keeps DVE free for exp
                nc.gpsimd.tensor_tensor(u_sb[t][:], st["avc"][:], bc_sb[:], MULT)
            else:
                st["bc_sb"] = bc_sb  # pair 2: DVE multiply, emitted later

        # ---- one attention block: sT(yt) + exp, av(yt-1) ----
        def att_block(t, yt):
            st = epi.setdefault(t, {"p": [[[None] * 2 for _ in range(8)] for _ in range(2)]})
            p_half = st["p"]
            if yt < 8:
                for j in range(2):
                    for s in range(2):
                        po, ke = (0, 48) if s == 0 else (64, 49)
                        hp = ps_h.tile([P, 512], F32, tag="h", name="sT_h")
                        nc.tensor.matmul(
                            hp[:],
                            lhsT=k_sb[po : po + ke, t, yt * 128 : (yt + 1) * 128],
                            rhs=q_sb[po : po + ke, t, j * 512 : (j + 1) * 512],
                            start=True,
                            stop=True,
                        )
                        if s == 0:
                            pt = ppool.tile([P, 512], BF16, tag="p", name="p_a")
                            nc.scalar.activation(pt[:], hp[:], EXP, bias=zb_sb[:])
                            p_half[s][yt][j] = pt[:]
                        else:
                            pt = ppool.tile([P, 512], U16, tag="p", name="p_d")
                            nc.vector.tensor_copy(pt[:], hp[:])
                            p_half[s][yt][j] = pt[:].bitcast(BF16)
            if yt > 0:
                if yt == 1:
                    st["av"] = ps_av.tile([P, SEQ], F32, tag="av", name=f"av{t}")
                for j in range(2):
                    for s in range(2):
                        po = s * 64
                        h = 2 * t + s
                        nc.tensor.matmul(
                            st["av"][po : po + VP, j * 512 : (j + 1) * 512],
                            lhsT=vT_sb[:, yt - 1, h, :],
                            rhs=p_half[s][yt - 1][j],
                            start=(yt == 1),
                            stop=(yt == 8),
                            skip_group_check=True,
                        )
            if yt == 8:
                # drain av to SBUF in halves (DVE + ACT), freeing the av
                # psum slot for the next pair
                avc = avcp.tile([P, SEQ], F32, tag="avc", name=f"avc{t}")
                nc.vector.tensor_copy(avc[:, 0:512], st["av"][:, 0:512])
                nc.scalar.activation(
                    avc[:, 512:SEQ], st["av"][:, 512:SEQ], IDENT, bias=zb_sb[:]
                )
                st["avc"] = avc

        # ---- the schedule: att blocks paced by the exp engines, with
        # qkv groups sprinkled into the PE's slack. Pair 0's q/k go
        # first; v groups early in pair 0 (av(t0,1) needs vT(0));
        # q1/k1 mid-pair-0, q2/k2 + q3/k3 across pair 1. ----
        for which in ("q", "k"):
            for j in range(2):
                qk_group(0, which, j)

        filler = {
            (0, 0): lambda: (v_group(0), v_group(1)),
            (0, 1): lambda: (v_group(2), v_group(3)),
            (0, 2): lambda: qk_group(1, "q", 0),
            (0, 3): lambda: qk_group(1, "q", 1),
            (0, 4): lambda: (v_group(4), v_group(5)),
            (0, 5): lambda: qk_group(1, "k", 0),
            (0, 6): lambda: (v_group(6), v_group(7)),
            (0, 7): lambda: qk_group(1, "k", 1),
            (1, 0): lambda: qk_group(2, "q", 0),
            (1, 1): lambda: qk_group(2, "q", 1),
            (1, 2): lambda: qk_group(2, "k", 0),
            (1, 3): lambda: qk_group(2, "k", 1),
            (1, 4): lambda: qk_group(3, "q", 0),
            (1, 5): lambda: qk_group(3, "q", 1),
            (1, 6): lambda: qk_group(3, "k", 0),
            (1, 7): lambda: qk_group(3, "k", 1),
        }
        # epilogue hooks: (pair being processed, yt) -> action on a
        # PREVIOUS pair. Pair 2's chain runs one yt earlier and
        # normalizes on DVE (its broadcast lands after exp is done).
        hooks = {
            (1, 1): lambda: epi_den(0),
            (1, 3): lambda: epi_recip(0),
            (1, 4): lambda: epi_bcast(0, "gpsimd"),
            (2, 1): lambda: epi_den(1),
            (2, 3): lambda: epi_recip(1),
            (2, 4): lambda: epi_bcast(1, "gpsimd"),
            (3, 0): lambda: epi_den(2),
            (3, 2): lambda: epi_recip(2),
            (3, 3): lambda: epi_bcast(2, "dve"),
        }

        for t in range(4):
            for yt in range(9):
                att_block(t, yt)
                f = filler.get((t, yt))
                if f:
                    f()
                h = hooks.get((t, yt))
                if h:
                    h()

        # ---- tail: fin(t3) + pair-2 DVE multiply + proj (kt-outer) ----
        # fin: 65-lane reciprocal straight off av_copy rows 0..64 (rows
        # 1..63 are junk but harmless), no DRAM bounce.
        avc3 = epi[3]["avc"]
        rec65 = rpool.tile([65, SEQ], BF16, tag="rec65")
        with nc.allow_low_precision(reason="softmax denom reciprocal to bf16"):
            nc.vector.reciprocal(rec65[:], avc3[0:65, :])
        # pair 2 normalize on DVE (emitted after fin recip so the DVE
        # FIFO never blocks the fin chain on pair 2's broadcast DMA)
        nc.vector.tensor_tensor(u_sb[2][:], epi[2]["avc"][:], epi[2]["bc_sb"][:], MULT)

        # proj kt rounds 0-2 (kt0/kt1 start while pair 2/3 epilogues run)
        prh = [
            [ps_h.tile([P, 512], F32, tag="h", name=f"pr{ot}{j}") for j in range(2)]
            for ot in range(3)
        ]
        for kt in range(3):
            for ot in range(3):
                for j in range(2):
                    nc.tensor.matmul(
                        prh[ot][j][:],
                        lhsT=wp_sb[:, kt, ot * 128 : (ot + 1) * 128],
                        rhs=u_sb[kt][:, j * 512 : (j + 1) * 512],
                        start=(kt == 0),
                        stop=False,
                        skip_group_check=True,
                    )

        # final-pair broadcast: K=1 ones-matmuls into the freed av slot
        # (rows 0/64 of rec65 -> psum rows 0:64 / 64:128), then the DVE
        # multiply lands u3 just before kt3 needs it.
        bc_ps = ps_av.tile([P, SEQ], F32, tag="av", name="bc_ps")
        for s in range(2):
            sp = 64 * s
            for j in range(2):
                nc.tensor.matmul(
                    bc_ps[s * 64 : (s + 1) * 64, j * 512 : (j + 1) * 512],
                    lhsT=ones_sb[sp : sp + 1, :],
                    rhs=rec65[sp : sp + 1, j * 512 : (j + 1) * 512],
                    start=True,
                    stop=True,
                )
        nc.vector.tensor_tensor(u_sb[3][:], avc3[:], bc_ps[:], MULT)

        for ot in range(3):
            for j in range(2):
                nc.tensor.matmul(
                    prh[ot][j][:],
                    lhsT=wp_sb[:, 3, ot * 128 : (ot + 1) * 128],
                    rhs=u_sb[3][:, j * 512 : (j + 1) * 512],
                    start=False,
                    stop=True,
                    skip_group_check=True,
                )

        oq = [nc.sync, nc.gpsimd, nc.scalar]
        for ot in range(3):
            o_sb = opool.tile([P, SEQ], BF16, tag="o")
            # bias-copy halves spread over ACT/DVE so the three tiles
            # drain in parallel instead of serializing on ACT
            for j in range(2):
                src = prh[ot][j][:]
                dst = o_sb[:, j * 512 : (j + 1) * 512]
                if (ot + j) % 2 == 1:
                    nc.vector.tensor_scalar(
                        dst, src, bias_sb[:, ot : ot + 1], None, ADD
                    )
                else:
                    nc.scalar.activation(
                        dst, src, IDENT, bias=bias_sb[:, ot : ot + 1]
                    )
            oq[ot].dma_start(out_d.ap()[ot * 128 : (ot + 1) * 128, :], o_sb[:])


def build_nc():
    nc = bacc.Bacc("TRN2", target_bir_lowering=False, debug=False, num_devices=NCORES)
    x_d = nc.dram_tensor("x", [P, 3, SEQ], BF16, kind="ExternalInput")
    wq0_d = nc.dram_tensor("wq0", [P, 512], BF16, kind="ExternalInput")
    wq12_d = nc.dram_tensor("wq12", [P, 2, 512], BF16, kind="ExternalInput")
    wk0_d = nc.dram_tensor("wk0", [P, 512], BF16, kind="ExternalInput")
    wk12_d = nc.dram_tensor("wk12", [P, 2, 512], BF16, kind="ExternalInput")
    wv_d = nc.dram_tensor("wv", [P, 3, HEADS * VP], BF16, kind="ExternalInput")
    wp_d = nc.dram_tensor("wp", [P, 4, DIM], BF16, kind="ExternalInput")
    b_d = nc.dram_tensor("bias", [P, 3], F32, kind="ExternalInput")
    brow_d = nc.dram_tensor("brow", [2, 4, SEQ], BF16, kind="ExternalInput")
    out_d = nc.dram_tensor("out", [DIM, SEQ], BF16, kind="ExternalOutput")

    with tile.TileContext(nc) as tc:
        _emit(tc, nc, x_d, wq0_d, wq12_d, wk0_d, wk12_d, wv_d, wp_d, b_d, brow_d, out_d)
    nc.compile()
    return nc


def pack_inputs(x, w_qkv, w_proj, b_proj):
    """Host-side weight packing. Returns per-core input maps."""
    import ml_dtypes

    x = np.asarray(x, np.float32)
    w_qkv = np.asarray(w_qkv, np.float32)
    w_proj = np.asarray(w_proj, np.float32)
    b_proj = np.asarray(b_proj, np.float32)
    scale = DH ** -0.5
    w_q, w_k, w_v = w_qkv[0:DIM], w_qkv[DIM : 2 * DIM], w_qkv[2 * DIM :]

    WQ = np.zeros((DIM, 512), np.float32)
    WK = np.zeros((DIM, 512), np.float32)
    WV = np.zeros((DIM, HEADS * VP), np.float32)
    WP = np.zeros((512, DIM), np.float32)
    for h in range(HEADS):
        col = (h // 2) * 128 + (h % 2) * 64
        # odd heads run the DVE Schraudolph path: fold A into the scale
        qs = scale * (A_EXP if h % 2 == 1 else 1.0)
        WQ[:, col : col + DH] = (w_q[h * DH : (h + 1) * DH] * qs).T
        WK[:, col : col + DH] = w_k[h * DH : (h + 1) * DH].T
        WV[:, h * VP + 16 : h * VP + 16 + DH] = w_v[h * DH : (h + 1) * DH].T
        WP[col + 16 : col + 16 + DH, :] = w_proj[:, h * DH : (h + 1) * DH].T
    BIAS = np.ascontiguousarray(b_proj.reshape(3, P).T)

    def pm(a, chunks):
        # [(chunks*P), f] -> [P, chunks, f] partition-major bf16 pre-layout
        return np.ascontiguousarray(
            a.reshape(chunks, P, a.shape[-1]).transpose(1, 0, 2)
        ).astype(ml_dtypes.bfloat16)

    WQp, WKp, WVp, WPp = pm(WQ, 3), pm(WK, 3), pm(WV, 3), pm(WP, 4)
    WQ0 = np.ascontiguousarray(WQp[:, 0, :])
    WQ12 = np.ascontiguousarray(WQp[:, 1:3, :])
    WK0 = np.ascontiguousarray(WKp[:, 0, :])
    WK12 = np.ascontiguousarray(WKp[:, 1:3, :])
    BROW = np.empty((2, 4, SEQ), np.float32)
    BROW[0] = 1.0
    BROW[1] = B_EXP
    BROW = BROW.astype(ml_dtypes.bfloat16)
    in_maps = []
    for b in range(NCORES):
        in_maps.append(
            {
                "x": pm(x[b].reshape(DIM, SEQ), 3),
                "wq0": WQ0,
                "wq12": WQ12,
                "wk0": WK0,
                "wk12": WK12,
                "wv": WVp,
                "wp": WPp,
                "bias": BIAS,
                "brow": BROW,
            }
        )
    return in_maps


def run(in_maps, trace=False):
    if "nc" not in _NC_CACHE:
        _NC_CACHE["nc"] = build_nc()
    nc = _NC_CACHE["nc"]
    res = run_bass_kernel_spmd(
        nc, in_maps, core_ids=list(range(NCORES)), trace=trace
    )
    out = np.stack(
        [res.results[i]["out"].astype(np.float32) for i in range(NCORES)]
    )
    return out.reshape(NCORES, DIM, 32, 32), res


def kernel(x, w_qkv, w_proj, b_proj):
    out, _ = run(pack_inputs(x, w_qkv, w_proj, b_proj))
    return out


# revision 15
# speedup vs baseline: 1.1527x; 1.0162x over previous
"""Trainium2 Bass kernel for nn_Attention_57166014709861.

8-batch image attention (B=8, C=384, h=8, d=48, HW=1024), data-parallel:
one batch image per NeuronCore, weights broadcast, host-side gather.

v3 (from v2 107us, v1 119-129us): the whole kernel is ONE software
pipeline paced by the exp engines (ACT+DVE), which own the critical
path (~74us of psum->sbuf work split across two engines).

  * Skewed emission: attention blocks att(t,yt) start right after pair
    0's q/k land (~15us instead of ~33us); the remaining qkv groups
    (q1-3/k1-3, v) are sprinkled one-per-yt into the attention stream so
    the PE (which has ~50% slack at the exp-bound pace) computes them in
    the gaps and HAM stays warm.
  * PSUM in single-bank [128,512] halves (6 rotating "h" slots shared by
    qkv/v/sT/proj): paired matmuls run CONCURRENTLY on disjoint PE
    sub-arrays (row groups 0/64 for the K<=49 sT, col groups 0/64 for
    the M=64 av) -- tile_position auto-derives from base partitions.
  * Static per-head exp split: even heads (s=0, partitions 0:48) use ACT
    native Exp; odd heads (s=1, partitions 64:113) use DVE. For DVE
    heads the Schraudolph affine map x = A*s + B is folded INTO the sT
    matmul (A rides the host-packed w_q scale, B rides contraction row
    112: q row 112 = 1, k row 112 = B, delivered by a tiny dep-free DMA
    since engines cannot address base partition 112; the q/k copies only
    write partitions 0:112 so the rows survive). DVE then runs a pure
    copy-convert f32->u16: trunc(x) viewed as bf16 bits == bf16(exp(s)),
    zero-mean +-4% err. bf16 rounds B to 16256 -- an integer shift of B
    is an exact constant factor on p that cancels in the normalization.
    Softmax-argument noise does NOT average away, so B must stay
    zero-mean-calibrated and anything coarser than bf16 on the q/k path
    blows the 2e-2 gate (fp8 measured 4.9e-2).
  * Epilogue re-phased to avoid engine FIFO head-of-line blocking: the
    denominator-scatter DMAs fire at the next pair's yt1/yt0, the DVE
    reciprocal only at yt3/yt2 (its dep already landed -> no stall), the
    broadcast DMA at yt4/yt3. Pairs 0/1 normalize on GPSIMD; pair 2 on
    DVE (its broadcast lands after exp is done, and GPSIMD's queue would
    be blocked); pair 3 skips the DMA bounce entirely: 65-lane DVE
    reciprocal straight off av_copy rows 0..64, K=1 ones-matmul
    broadcast on the PE, DVE multiply from PSUM.
  * proj is kt-outer (all three output tiles' kt round together) so each
    round starts the moment its u tile lands; kt3 lands last after the
    final pair's u3.
"""

import sys

if "/opt/trn_rl_repo" not in sys.path:
    sys.path.insert(0, "/opt/trn_rl_repo")

import numpy as np

import concourse.bass as bass
import concourse.mybir as mybir
import concourse.tile as tile
from concourse import bacc
from concourse.bass_utils import run_bass_kernel_spmd

DIM = 384
HEADS = 8
DH = 48
SEQ = 1024
P = 128
NCORES = 8
VP = 64  # packed v cols per head: ones at col 0, zeros 1-15, 48 data at 16-63

F32 = mybir.dt.float32
BF16 = mybir.dt.bfloat16
U16 = mybir.dt.uint16
EXP = mybir.ActivationFunctionType.Exp
IDENT = mybir.ActivationFunctionType.Identity
ADD = mybir.AluOpType.add
MULT = mybir.AluOpType.mult

# Schraudolph exp-as-bf16-bits: trunc(s*A + B) viewed as bf16 ~= exp(s).
# A = 128/ln2; B calibrated for ZERO-MEAN rel err (max 4.2%, rms 1.8%)
# under truncation.
A_EXP = 184.6649652337873
B_EXP = 16249.125

_NC_CACHE = {}


def _emit(tc, nc, x_d, wq0_d, wq12_d, wk0_d, wk12_d, wv_d, wp_d, b_d, brow_d, out_d):
    with (
        tc.tile_pool(name="const", bufs=1) as constp,
        tc.tile_pool(name="weights", bufs=1) as wpool,
        tc.tile_pool(name="data", bufs=1) as data,
        tc.tile_pool(name="ptile", bufs=16) as ppool,
        tc.tile_pool(name="bcpool", bufs=3) as bcpool,
        tc.tile_pool(name="rpool", bufs=3) as rpool,
        tc.tile_pool(name="avcp", bufs=3) as avcp,
        tc.tile_pool(name="opool", bufs=3) as opool,
        tc.tile_pool(name="ps_h", bufs=6, space="PSUM") as ps_h,
        tc.tile_pool(name="ps_av", bufs=1, space="PSUM") as ps_av,
        tc.tile_pool(name="dram", bufs=3, space="DRAM") as drampool,
    ):
        # ---- loads (bf16, 3 queues). Dependency tracking is per-tile
        # (with subtile refinement): x per ko chunk, wq/wk split ko0 vs
        # ko12 so pair 0's q/k matmuls only wait for x+wq0/wk0. x2 rides
        # the scalar queue so all of x lands by ~13us (q0/k0 accumulate
        # all three ko chunks before the first sT can go).
        x_ch = [
            data.tile([P, SEQ], BF16, tag=f"x{ko}", name=f"x{ko}")
            for ko in range(3)
        ]
        wq0_sb = wpool.tile([P, 512], BF16, tag="wq0")
        wq12_sb = wpool.tile([P, 2, 512], BF16, tag="wq12")
        wk0_sb = wpool.tile([P, 512], BF16, tag="wk0")
        wk12_sb = wpool.tile([P, 2, 512], BF16, tag="wk12")
        wv_sb = wpool.tile([P, 3, HEADS * VP], BF16, tag="wv")
        wp_sb = wpool.tile([P, 4, DIM], BF16, tag="wp")
        bias_sb = constp.tile([P, 3], F32, tag="bias")

        q_sb = data.tile([P, 4, SEQ], BF16, tag="q")
        k_sb = data.tile([P, 4, SEQ], BF16, tag="k")
        vT_sb = data.tile([P, 8, HEADS, VP], BF16, tag="vT")

        nc.sync.dma_start(x_ch[0][:], x_d.ap()[:, 0, :])
        nc.scalar.dma_start(wq0_sb[:], wq0_d.ap())
        nc.gpsimd.dma_start(wk0_sb[:], wk0_d.ap())
        # Schraudolph affine rows (dep-free: q/k copies never write
        # partition 112+): q row 112 = 1, k row 112 = B.
        nc.gpsimd.dma_start(q_sb[112:113, :, :], brow_d.ap()[0:1, :, :])
        nc.gpsimd.dma_start(k_sb[112:113, :, :], brow_d.ap()[1:2, :, :])
        nc.sync.dma_start(x_ch[1][:], x_d.ap()[:, 1, :])
        nc.scalar.dma_start(x_ch[2][:], x_d.ap()[:, 2, :])
        nc.gpsimd.dma_start(wk12_sb[:], wk12_d.ap())
        nc.scalar.dma_start(wq12_sb[:], wq12_d.ap())
        nc.gpsimd.dma_start(wv_sb[:], wv_d.ap())
        nc.gpsimd.dma_start(wp_sb[:], wp_d.ap())
        nc.scalar.dma_start(bias_sb[:], b_d.ap())

        def wq_ko(ko):
            return wq0_sb[:] if ko == 0 else wq12_sb[:, ko - 1, :]

        def wk_ko(ko):
            return wk0_sb[:] if ko == 0 else wk12_sb[:, ko - 1, :]

        zb_sb = constp.tile([P, 1], F32, tag="zb")
        nc.gpsimd.memset(zb_sb[:], 0.0)
        # rows 0 and 64 are used as K=1 matmul lhsT for the final-pair
        # broadcast, so the ones tile spans 65 partitions.
        ones_sb = constp.tile([65, 64], BF16, tag="ones")
        nc.gpsimd.memset(ones_sb[:], 1.0)

        # ---- copy engine rotation (GPSIMD cannot read PSUM) ----
        _ncopy = [0]

        def eng_copy(dst, src):
            if _ncopy[0] % 2 == 0:
                nc.vector.tensor_copy(dst, src)
            else:
                nc.scalar.activation(
                    dst, src, IDENT, bias=zb_sb[0 : src.shape[0], :]
                )
            _ncopy[0] += 1

        # ---- qkv work groups (emitted interleaved into the attention
        # stream below). q/k copies write partitions 0:112 only, so the
        # Schraudolph rows survive. ----
        def qk_group(t, which, j):
            dst, wf = (q_sb, wq_ko) if which == "q" else (k_sb, wk_ko)
            ps = ps_h.tile([P, 512], F32, tag="h", name="qk_ps")
            for ko in range(3):
                nc.tensor.matmul(
                    ps[:],
                    lhsT=wf(ko)[:, t * 128 : (t + 1) * 128],
                    rhs=x_ch[ko][:, j * 512 : (j + 1) * 512],
                    start=(ko == 0),
                    stop=(ko == 2),
                )
            eng_copy(dst[0:112, t, j * 512 : (j + 1) * 512], ps[0:112, :])

        def v_group(yt):
            ps = ps_h.tile([P, 512], F32, tag="h", name="v_ps")
            for ko in range(3):
                nc.tensor.matmul(
                    ps[:],
                    lhsT=x_ch[ko][:, yt * 128 : (yt + 1) * 128],
                    rhs=wv_sb[:, ko, :],
                    start=(ko == 0),
                    stop=(ko == 2),
                )
            eng_copy(
                vT_sb[:, yt, :, :],
                ps[:].rearrange("p (h v) -> p h v", h=HEADS),
            )
            # ones column (col 0) for the softmax denominator (psum row 0/64)
            nc.gpsimd.memset(vT_sb[:, yt, :, 0:1], 1.0)

        # ---- attention state ----
        u_sb = [data.tile([P, SEQ], BF16, tag=f"u{i}", name=f"u{i}") for i in range(4)]
        epi = {}  # pair -> dict with av_copy / den_dram / rec_dram

        # Epilogue for pairs 0-2, phase-split so no engine FIFO-blocks on
        # a DMA that hasn't landed: den scatter DMAs early, reciprocal
        # two yts later (dep already met), broadcast next.
        def epi_den(t):
            st = epi[t]
            den_dram = drampool.tile([2 * SEQ], F32, tag="den")
            nc.sync.dma_start(den_dram[0:SEQ], st["avc"][0:1, :])
            nc.sync.dma_start(den_dram[SEQ : 2 * SEQ], st["avc"][64:65, :])
            den_pm = rpool.tile([P, 16], F32, tag="denpm")
            nc.sync.dma_start(den_pm[:], den_dram[:].rearrange("(p f) -> p f", p=P))
            st["den_pm"] = den_pm

        def epi_recip(t):
            st = epi[t]
            rec_pm = rpool.tile([P, 16], BF16, tag="recpm")
            with nc.allow_low_precision(reason="softmax denom reciprocal to bf16"):
                nc.vector.reciprocal(rec_pm[:], st["den_pm"][:])
            rec_dram = drampool.tile([2 * SEQ], BF16, tag="rec")
            nc.sync.dma_start(rec_dram[:], rec_pm[:])
            st["rec_dram"] = rec_dram

        def epi_bcast(t, mult_engine):
            st = epi[t]
            bc_sb = bcpool.tile([P, SEQ], BF16, tag="bcs")
            rec_r = st["rec_dram"][:].rearrange("(h f) -> h f", h=2)
            nc.sync.dma_start(bc_sb[:], rec_r[:, None, :].to_broadcast([2, 64, SEQ]))
            if mult_engine == "gpsimd":
                # all-SBUF multiply on Pool keeps DVE free for exp
                nc.gpsimd.tensor_tensor(u_sb[t][:], st["avc"][:], bc_sb[:], MULT)
            else:
                st["bc_sb"] = bc_sb  # pair 2: DVE multiply, emitted later

        # ---- one attention block: sT(yt) + exp, av(yt-1) ----
        def att_block(t, yt):
            st = epi.setdefault(t, {"p": [[[None] * 2 for _ in range(8)] for _ in range(2)]})
            p_half = st["p"]
            if yt < 8:
                for j in range(2):
                    for s in range(2):
                        po, ke = (0, 48) if s == 0 else (64, 49)
                        hp = ps_h.tile([P, 512], F32, tag="h", name="sT_h")
                        nc.tensor.matmul(
                            hp[:],
                            lhsT=k_sb[po : po + ke, t, yt * 128 : (yt + 1) * 128],
                            rhs=q_sb[po : po + ke, t, j * 512 : (j + 1) * 512],
                            start=True,
                            stop=True,
                        )
                        if s == 0:
                            pt = ppool.tile([P, 512], BF16, tag="p", name="p_a")
                            nc.scalar.activation(pt[:], hp[:], EXP, bias=zb_sb[:])
                            p_half[s][yt][j] = pt[:]
                        else:
                            pt = ppool.tile([P, 512], U16, tag="p", name="p_d")
                            nc.vector.tensor_copy(pt[:], hp[:])
                            p_half[s][yt][j] = pt[:].bitcast(BF16)
            if yt > 0:
                if yt == 1:
                    st["av"] = ps_av.tile([P, SEQ], F32, tag="av", name=f"av{t}")
                for j in range(2):
                    for s in range(2):
                        po = s * 64
                        h = 2 * t + s
                        nc.tensor.matmul(
                            st["av"][po : po + VP, j * 512 : (j + 1) * 512],
                            lhsT=vT_sb[:, yt - 1, h, :],
                            rhs=p_half[s][yt - 1][j],
                            start=(yt == 1),
                            stop=(yt == 8),
                            skip_group_check=True,
                        )
            if yt == 8:
                # drain av to SBUF in halves (DVE + ACT), freeing the av
                # psum slot for the next pair
                avc = avcp.tile([P, SEQ], F32, tag="avc", name=f"avc{t}")
                nc.vector.tensor_copy(avc[:, 0:512], st["av"][:, 0:512])
                nc.scalar.activation(
                    avc[:, 512:SEQ], st["av"][:, 512:SEQ], IDENT, bias=zb_sb[:]
                )
                st["avc"] = avc

        # ---- the schedule: att blocks paced by the exp engines, with
        # qkv groups sprinkled into the PE's slack. Pair 0's q/k go
        # first; v groups early in pair 0 (av(t0,1) needs vT(0));
        # q1/k1 mid-pair-0, q2/k2 + q3/k3 across pair 1. ----
        for which in ("q", "k"):
            for j in range(2):
                qk_group(0, which, j)

        filler = {
            (0, 0): lambda: (v_group(0), v_group(1)),
            (0, 1): lambda: (v_group(2), v_group(3)),
            (0, 2): lambda: qk_group(1, "q", 0),
            (0, 3): lambda: qk_group(1, "q", 1),
            (0, 4): lambda: (v_group(4), v_group(5)),
            (0, 5): lambda: qk_group(1, "k", 0),
            (0, 6): lambda: (v_group(6), v_group(7)),
            (0, 7): lambda: qk_group(1, "k", 1),
            (1, 0): lambda: qk_group(2, "q", 0),
            (1, 1): lambda: qk_group(2, "q", 1),
            (1, 2): lambda: qk_group(2, "k", 0),
            (1, 3): lambda: qk_group(2, "k", 1),
            (1, 4): lambda: qk_group(3, "q", 0),
            (1, 5): lambda: qk_group(3, "q", 1),
            (1, 6): lambda: qk_group(3, "k", 0),
            (1, 7): lambda: qk_group(3, "k", 1),
        }
        # epilogue hooks: (pair being processed, yt) -> action on a
        # PREVIOUS pair. Pair 2's chain runs one yt earlier and
        # normalizes on DVE (its broadcast lands after exp is done).
        hooks = {
            (1, 1): lambda: epi_den(0),
            (1, 3): lambda: epi_recip(0),
            (1, 4): lambda: epi_bcast(0, "gpsimd"),
            (2, 1): lambda: epi_den(1),
            (2, 3): lambda: epi_recip(1),
            (2, 4): lambda: epi_bcast(1, "gpsimd"),
            (3, 0): lambda: epi_den(2),
            (3, 1): lambda: epi_recip(2),
            (3, 2): lambda: epi_bcast(2, "gpsimd"),
        }

        for t in range(4):
            for yt in range(9):
                att_block(t, yt)
                f = filler.get((t, yt))
                if f:
                    f()
                h = hooks.get((t, yt))
                if h:
                    h()

        # ---- tail: fin(t3) + proj (kt-outer) ----
        # fin: scatter the denominator rows to 128 lanes (reciprocal is
        # ~6 cyc/element on DVE, so narrow-lane forms lose), reciprocal
        # in one 253ns op, scatter back to a row pair for the PE
        # ones-matmul broadcast.
        avc3 = epi[3]["avc"]
        den_pm3 = rpool.tile([P, 16], F32, tag="denpm")
        nc.sync.dma_start(den_pm3[:, 0:8], avc3[0:1, :])
        nc.scalar.dma_start(den_pm3[:, 8:16], avc3[64:65, :])
        rec_pm3 = rpool.tile([P, 16], BF16, tag="recpm")
        with nc.allow_low_precision(reason="softmax denom reciprocal to bf16"):
            nc.vector.reciprocal(rec_pm3[:], den_pm3[:])
        rec65 = rpool.tile([33, SEQ], BF16, tag="rec65")
        nc.sync.dma_start(rec65[0:1, :], rec_pm3[:, 0:8])
        nc.scalar.dma_start(rec65[32:33, :], rec_pm3[:, 8:16])

        # proj kt rounds 0-2 (kt0/kt1 start while pair 2/3 epilogues run)
        prh = [
            [ps_h.tile([P, 512], F32, tag="h", name=f"pr{ot}{j}") for j in range(2)]
            for ot in range(3)
        ]
        for kt in range(3):
            for ot in range(3):
                for j in range(2):
                    nc.tensor.matmul(
                        prh[ot][j][:],
                        lhsT=wp_sb[:, kt, ot * 128 : (ot + 1) * 128],
                        rhs=u_sb[kt][:, j * 512 : (j + 1) * 512],
                        start=(kt == 0),
                        stop=False,
                        skip_group_check=True,
                    )

        # final-pair broadcast: K=1 ones-matmuls into the freed av slot
        # (rows 0/64 of rec65 -> psum rows 0:64 / 64:128), then the DVE
        # multiply lands u3 just before kt3 needs it.
        bc_ps = ps_av.tile([P, SEQ], F32, tag="av", name="bc_ps")
        for s in range(2):
            sp = 32 * s
            for j in range(2):
                nc.tensor.matmul(
                    bc_ps[s * 64 : (s + 1) * 64, j * 512 : (j + 1) * 512],
                    lhsT=ones_sb[sp : sp + 1, :],
                    rhs=rec65[sp : sp + 1, j * 512 : (j + 1) * 512],
                    start=True,
                    stop=True,
                )
        nc.vector.tensor_tensor(u_sb[3][:], avc3[:], bc_ps[:], MULT)

        for ot in range(3):
            for j in range(2):
                nc.tensor.matmul(
                    prh[ot][j][:],
                    lhsT=wp_sb[:, 3, ot * 128 : (ot + 1) * 128],
                    rhs=u_sb[3][:, j * 512 : (j + 1) * 512],
                    start=False,
                    stop=True,
                    skip_group_check=True,
                )

        oq = [nc.sync, nc.gpsimd, nc.scalar]
        for ot in range(3):
            o_sb = opool.tile([P, SEQ], BF16, tag="o")
            # bias-copy halves spread over ACT/DVE so the three tiles
            # drain in parallel instead of serializing on ACT
            for j in range(2):
                src = prh[ot][j][:]
                dst = o_sb[:, j * 512 : (j + 1) * 512]
                if (ot + j) % 2 == 1:
                    nc.vector.tensor_scalar(
                        dst, src, bias_sb[:, ot : ot + 1], None, ADD
                    )
                else:
                    nc.scalar.activation(
                        dst, src, IDENT, bias=bias_sb[:, ot : ot + 1]
                    )
            oq[ot].dma_start(out_d.ap()[ot * 128 : (ot + 1) * 128, :], o_sb[:])


def build_nc():
    nc = bacc.Bacc("TRN2", target_bir_lowering=False, debug=False, num_devices=NCORES)
    x_d = nc.dram_tensor("x", [P, 3, SEQ], BF16, kind="ExternalInput")
    wq0_d = nc.dram_tensor("wq0", [P, 512], BF16, kind="ExternalInput")
    wq12_d = nc.dram_tensor("wq12", [P, 2, 512], BF16, kind="ExternalInput")
    wk0_d = nc.dram_tensor("wk0", [P, 512], BF16, kind="ExternalInput")
    wk12_d = nc.dram_tensor("wk12", [P, 2, 512], BF16, kind="ExternalInput")
    wv_d = nc.dram_tensor("wv", [P, 3, HEADS * VP], BF16, kind="ExternalInput")
    wp_d = nc.dram_tensor("wp", [P, 4, DIM], BF16, kind="ExternalInput")
    b_d = nc.dram_tensor("bias", [P, 3], F32, kind="ExternalInput")
    brow_d = nc.dram_tensor("brow", [2, 4, SEQ], BF16, kind="ExternalInput")
    out_d = nc.dram_tensor("out", [DIM, SEQ], BF16, kind="ExternalOutput")

    with tile.TileContext(nc) as tc:
        _emit(tc, nc, x_d, wq0_d, wq12_d, wk0_d, wk12_d, wv_d, wp_d, b_d, brow_d, out_d)
    nc.compile()
    return nc


def pack_inputs(x, w_qkv, w_proj, b_proj):
    """Host-side weight packing. Returns per-core input maps."""
    import ml_dtypes

    x = np.asarray(x, np.float32)
    w_qkv = np.asarray(w_qkv, np.float32)
    w_proj = np.asarray(w_proj, np.float32)
    b_proj = np.asarray(b_proj, np.float32)
    scale = DH ** -0.5
    w_q, w_k, w_v = w_qkv[0:DIM], w_qkv[DIM : 2 * DIM], w_qkv[2 * DIM :]

    WQ = np.zeros((DIM, 512), np.float32)
    WK = np.zeros((DIM, 512), np.float32)
    WV = np.zeros((DIM, HEADS * VP), np.float32)
    WP = np.zeros((512, DIM), np.float32)
    for h in range(HEADS):
        col = (h // 2) * 128 + (h % 2) * 64
        # odd heads run the DVE Schraudolph path: fold A into the scale
        qs = scale * (A_EXP if h % 2 == 1 else 1.0)
        WQ[:, col : col + DH] = (w_q[h * DH : (h + 1) * DH] * qs).T
        WK[:, col : col + DH] = w_k[h * DH : (h + 1) * DH].T
        WV[:, h * VP + 16 : h * VP + 16 + DH] = w_v[h * DH : (h + 1) * DH].T
        WP[col + 16 : col + 16 + DH, :] = w_proj[:, h * DH : (h + 1) * DH].T
    BIAS = np.ascontiguousarray(b_proj.reshape(3, P).T)

    def pm(a, chunks):
        # [(chunks*P), f] -> [P, chunks, f] partition-major bf16 pre-layout
        return np.ascontiguousarray(
            a.reshape(chunks, P, a.shape[-1]).transpose(1, 0, 2)
        ).astype(ml_dtypes.bfloat16)

    WQp, WKp, WVp, WPp = pm(WQ, 3), pm(WK, 3), pm(WV, 3), pm(WP, 4)
    WQ0 = np.ascontiguousarray(WQp[:, 0, :])
    WQ12 = np.ascontiguousarray(WQp[:, 1:3, :])
    WK0 = np.ascontiguousarray(WKp[:, 0, :])
    WK12 = np.ascontiguousarray(WKp[:, 1:3, :])
    BROW = np.empty((2, 4, SEQ), np.float32)
    BROW[0] = 1.0
    BROW[1] = B_EXP
    BROW = BROW.astype(ml_dtypes.bfloat16)
    in_maps = []
    for b in range(NCORES):
        in_maps.append(
            {
                "x": pm(x[b].reshape(DIM, SEQ), 3),
                "wq0": WQ0,
                "wq12": WQ12,
                "wk0": WK0,
                "wk12": WK12,
                "wv": WVp,
                "wp": WPp,
                "bias": BIAS,
                "brow": BROW,
            }
        )
    return in_maps


def run(in_maps, trace=False):
    if "nc" not in _NC_CACHE:
        _NC_CACHE["nc"] = build_nc()
    nc = _NC_CACHE["nc"]
    res = run_bass_kernel_spmd(
        nc, in_maps, core_ids=list(range(NCORES)), trace=trace
    )
    out = np.stack(
        [res.results[i]["out"].astype(np.float32) for i in range(NCORES)]
    )
    return out.reshape(NCORES, DIM, 32, 32), res


def kernel(x, w_qkv, w_proj, b_proj):
    out, _ = run(pack_inputs(x, w_qkv, w_proj, b_proj))
    return out


# revision 18
# speedup vs baseline: 1.2127x; 1.0521x over previous
"""Trainium2 Bass kernel for nn_Attention_57166014709861.

8-batch image attention (B=8, C=384, h=8, d=48, HW=1024), data-parallel:
one batch image per NeuronCore, weights broadcast, host-side gather.

v3 (from v2 107us, v1 119-129us): the whole kernel is ONE software
pipeline paced by the exp engines (ACT+DVE), which own the critical
path (~74us of psum->sbuf work split across two engines).

  * Skewed emission: attention blocks att(t,yt) start right after pair
    0's q/k land (~15us instead of ~33us); the remaining qkv groups
    (q1-3/k1-3, v) are sprinkled one-per-yt into the attention stream so
    the PE (which has ~50% slack at the exp-bound pace) computes them in
    the gaps and HAM stays warm.
  * PSUM in single-bank [128,512] halves (6 rotating "h" slots shared by
    qkv/v/sT/proj): paired matmuls run CONCURRENTLY on disjoint PE
    sub-arrays (row groups 0/64 for the K<=49 sT, col groups 0/64 for
    the M=64 av) -- tile_position auto-derives from base partitions.
  * Static per-head exp split: even heads (s=0, partitions 0:48) use ACT
    native Exp; odd heads (s=1, partitions 64:113) use DVE. For DVE
    heads the Schraudolph affine map x = A*s + B is folded INTO the sT
    matmul (A rides the host-packed w_q scale, B rides contraction row
    112: q row 112 = 1, k row 112 = B, delivered by a tiny dep-free DMA
    since engines cannot address base partition 112; the q/k copies only
    write partitions 0:112 so the rows survive). DVE then runs a pure
    copy-convert f32->u16: trunc(x) viewed as bf16 bits == bf16(exp(s)),
    zero-mean +-4% err. bf16 rounds B to 16256 -- an integer shift of B
    is an exact constant factor on p that cancels in the normalization.
    Softmax-argument noise does NOT average away, so B must stay
    zero-mean-calibrated and anything coarser than bf16 on the q/k path
    blows the 2e-2 gate (fp8 measured 4.9e-2).
  * Epilogue re-phased to avoid engine FIFO head-of-line blocking: the
    denominator-scatter DMAs fire at the next pair's yt1/yt0, the DVE
    reciprocal only at yt3/yt2 (its dep already landed -> no stall), the
    broadcast DMA at yt4/yt3. Pairs 0/1 normalize on GPSIMD; pair 2 on
    DVE (its broadcast lands after exp is done, and GPSIMD's queue would
    be blocked); pair 3 skips the DMA bounce entirely: 65-lane DVE
    reciprocal straight off av_copy rows 0..64, K=1 ones-matmul
    broadcast on the PE, DVE multiply from PSUM.
  * proj is kt-outer (all three output tiles' kt round together) so each
    round starts the moment its u tile lands; kt3 lands last after the
    final pair's u3.
"""

import sys

if "/opt/trn_rl_repo" not in sys.path:
    sys.path.insert(0, "/opt/trn_rl_repo")

import numpy as np

import concourse.bass as bass
import concourse.mybir as mybir
import concourse.tile as tile
from concourse import bacc
from concourse.bass_utils import run_bass_kernel_spmd

DIM = 384
HEADS = 8
DH = 48
SEQ = 1024
P = 128
NCORES = 8
VP = 64  # packed v cols per head: ones at col 0, zeros 1-15, 48 data at 16-63

F32 = mybir.dt.float32
BF16 = mybir.dt.bfloat16
U16 = mybir.dt.uint16
EXP = mybir.ActivationFunctionType.Exp
IDENT = mybir.ActivationFunctionType.Identity
ADD = mybir.AluOpType.add
MULT = mybir.AluOpType.mult

# Schraudolph exp-as-bf16-bits: trunc(s*A + B) viewed as bf16 ~= exp(s).
# A = 128/ln2; B calibrated for ZERO-MEAN rel err (max 4.2%, rms 1.8%)
# under truncation.
A_EXP = 184.6649652337873
B_EXP = 16249.125

_NC_CACHE = {}


def _emit(tc, nc, x_d, wq0_d, wq12_d, wk0_d, wk12_d, wv_d, wp_d, b_d, brow_d, out_d):
    with (
        tc.tile_pool(name="const", bufs=1) as constp,
        tc.tile_pool(name="weights", bufs=1) as wpool,
        tc.tile_pool(name="data", bufs=1) as data,
        tc.tile_pool(name="ptile", bufs=16) as ppool,
        tc.tile_pool(name="bcpool", bufs=3) as bcpool,
        tc.tile_pool(name="rpool", bufs=3) as rpool,
        tc.tile_pool(name="avcp", bufs=3) as avcp,
        tc.tile_pool(name="opool", bufs=3) as opool,
        tc.tile_pool(name="ps_h", bufs=6, space="PSUM") as ps_h,
        tc.tile_pool(name="ps_av", bufs=1, space="PSUM") as ps_av,
        tc.tile_pool(name="dram", bufs=3, space="DRAM") as drampool,
    ):
        # ---- loads (bf16, 3 queues). Dependency tracking is per-tile
        # (with subtile refinement): x per ko chunk, wq/wk split ko0 vs
        # ko12 so pair 0's q/k matmuls only wait for x+wq0/wk0. x2 rides
        # the scalar queue so all of x lands by ~13us (q0/k0 accumulate
        # all three ko chunks before the first sT can go).
        x_ch = [
            data.tile([P, SEQ], BF16, tag=f"x{ko}", name=f"x{ko}")
            for ko in range(3)
        ]
        wq0_sb = wpool.tile([P, 512], BF16, tag="wq0")
        wq12_sb = wpool.tile([P, 2, 512], BF16, tag="wq12")
        wk0_sb = wpool.tile([P, 512], BF16, tag="wk0")
        wk12_sb = wpool.tile([P, 2, 512], BF16, tag="wk12")
        wv_sb = wpool.tile([P, 3, HEADS * VP], BF16, tag="wv")
        wp_sb = wpool.tile([P, 4, DIM], BF16, tag="wp")
        bias_sb = constp.tile([P, 3], F32, tag="bias")

        q_sb = data.tile([P, 4, SEQ], BF16, tag="q")
        k_sb = data.tile([P, 4, SEQ], BF16, tag="k")
        vT_sb = data.tile([P, 8, HEADS, VP], BF16, tag="vT")

        nc.sync.dma_start(x_ch[0][:], x_d.ap()[:, 0, :])
        nc.scalar.dma_start(wq0_sb[:], wq0_d.ap())
        nc.gpsimd.dma_start(wk0_sb[:], wk0_d.ap())
        # Schraudolph affine rows (dep-free: q/k copies never write
        # partition 112+): q row 112 = 1, k row 112 = B.
        nc.gpsimd.dma_start(q_sb[112:113, :, :], brow_d.ap()[0:1, :, :])
        nc.gpsimd.dma_start(k_sb[112:113, :, :], brow_d.ap()[1:2, :, :])
        nc.sync.dma_start(x_ch[1][:], x_d.ap()[:, 1, :])
        nc.scalar.dma_start(x_ch[2][:], x_d.ap()[:, 2, :])
        nc.gpsimd.dma_start(wk12_sb[:], wk12_d.ap())
        nc.scalar.dma_start(wq12_sb[:], wq12_d.ap())
        nc.gpsimd.dma_start(wv_sb[:], wv_d.ap())
        nc.gpsimd.dma_start(wp_sb[:], wp_d.ap())
        nc.scalar.dma_start(bias_sb[:], b_d.ap())

        def wq_ko(ko):
            return wq0_sb[:] if ko == 0 else wq12_sb[:, ko - 1, :]

        def wk_ko(ko):
            return wk0_sb[:] if ko == 0 else wk12_sb[:, ko - 1, :]

        zb_sb = constp.tile([P, 1], F32, tag="zb")
        nc.gpsimd.memset(zb_sb[:], 0.0)
        # rows 0 and 64 are used as K=1 matmul lhsT for the final-pair
        # broadcast, so the ones tile spans 65 partitions.
        ones_sb = constp.tile([65, 64], BF16, tag="ones")
        nc.gpsimd.memset(ones_sb[:], 1.0)

        # ---- copy engine rotation (GPSIMD cannot read PSUM) ----
        _ncopy = [0]

        def eng_copy(dst, src):
            if _ncopy[0] % 2 == 0:
                nc.vector.tensor_copy(dst, src)
            else:
                nc.scalar.activation(
                    dst, src, IDENT, bias=zb_sb[0 : src.shape[0], :]
                )
            _ncopy[0] += 1

        # ---- qkv work groups (emitted interleaved into the attention
        # stream below). q/k copies write partitions 0:112 only, so the
        # Schraudolph rows survive. ----
        def qk_group(t, which, j):
            dst, wf = (q_sb, wq_ko) if which == "q" else (k_sb, wk_ko)
            ps = ps_h.tile([P, 512], F32, tag="h", name="qk_ps")
            for ko in range(3):
                nc.tensor.matmul(
                    ps[:],
                    lhsT=wf(ko)[:, t * 128 : (t + 1) * 128],
                    rhs=x_ch[ko][:, j * 512 : (j + 1) * 512],
                    start=(ko == 0),
                    stop=(ko == 2),
                )
            eng_copy(dst[0:112, t, j * 512 : (j + 1) * 512], ps[0:112, :])

        def v_group(yt):
            ps = ps_h.tile([P, 512], F32, tag="h", name="v_ps")
            for ko in range(3):
                nc.tensor.matmul(
                    ps[:],
                    lhsT=x_ch[ko][:, yt * 128 : (yt + 1) * 128],
                    rhs=wv_sb[:, ko, :],
                    start=(ko == 0),
                    stop=(ko == 2),
                )
            eng_copy(
                vT_sb[:, yt, :, :],
                ps[:].rearrange("p (h v) -> p h v", h=HEADS),
            )
            # ones column (col 0) for the softmax denominator (psum row 0/64)
            nc.gpsimd.memset(vT_sb[:, yt, :, 0:1], 1.0)

        # ---- attention state ----
        u_sb = [data.tile([P, SEQ], BF16, tag=f"u{i}", name=f"u{i}") for i in range(4)]
        epi = {}  # pair -> dict with av_copy / den_dram / rec_dram

        # Epilogue for pairs 0-2, phase-split so no engine FIFO-blocks on
        # a DMA that hasn't landed: den scatter DMAs early, reciprocal
        # two yts later (dep already met), broadcast next.
        def epi_den(t):
            st = epi[t]
            den_dram = drampool.tile([2 * SEQ], F32, tag="den")
            nc.sync.dma_start(den_dram[0:SEQ], st["avc"][0:1, :])
            nc.sync.dma_start(den_dram[SEQ : 2 * SEQ], st["avc"][64:65, :])
            den_pm = rpool.tile([P, 16], F32, tag="denpm")
            nc.sync.dma_start(den_pm[:], den_dram[:].rearrange("(p f) -> p f", p=P))
            st["den_pm"] = den_pm

        def epi_recip(t):
            st = epi[t]
            rec_pm = rpool.tile([P, 16], BF16, tag="recpm")
            with nc.allow_low_precision(reason="softmax denom reciprocal to bf16"):
                nc.vector.reciprocal(rec_pm[:], st["den_pm"][:])
            rec_dram = drampool.tile([2 * SEQ], BF16, tag="rec")
            nc.sync.dma_start(rec_dram[:], rec_pm[:])
            st["rec_dram"] = rec_dram

        def epi_bcast(t, mult_engine):
            st = epi[t]
            bc_sb = bcpool.tile([P, SEQ], BF16, tag="bcs")
            rec_r = st["rec_dram"][:].rearrange("(h f) -> h f", h=2)
            nc.sync.dma_start(bc_sb[:], rec_r[:, None, :].to_broadcast([2, 64, SEQ]))
            if mult_engine == "gpsimd":
                # all-SBUF multiply on Pool keeps DVE free for exp
                nc.gpsimd.tensor_tensor(u_sb[t][:], st["avc"][:], bc_sb[:], MULT)
            else:
                st["bc_sb"] = bc_sb  # pair 2: DVE multiply, emitted later

        # ---- one attention block: sT(yt) + exp, av(yt-1) ----
        def att_block(t, yt):
            st = epi.setdefault(t, {"p": [[[None] * 2 for _ in range(8)] for _ in range(2)]})
            p_half = st["p"]
            if yt < 8:
                for j in range(2):
                    for s in range(2):
                        po, ke = (0, 48) if s == 0 else (64, 49)
                        hp = ps_h.tile([P, 512], F32, tag="h", name="sT_h")
                        nc.tensor.matmul(
                            hp[:],
                            lhsT=k_sb[po : po + ke, t, yt * 128 : (yt + 1) * 128],
                            rhs=q_sb[po : po + ke, t, j * 512 : (j + 1) * 512],
                            start=True,
                            stop=True,
                        )
                        if s == 0:
                            pt = ppool.tile([P, 512], BF16, tag="p", name="p_a")
                            nc.scalar.activation(pt[:], hp[:], EXP, bias=zb_sb[:])
                            p_half[s][yt][j] = pt[:]
                        else:
                            pt = ppool.tile([P, 512], U16, tag="p", name="p_d")
                            nc.vector.tensor_copy(pt[:], hp[:])
                            p_half[s][yt][j] = pt[:].bitcast(BF16)
            if yt > 0:
                if yt == 1:
                    st["av"] = ps_av.tile([P, SEQ], F32, tag="av", name=f"av{t}")
                for j in range(2):
                    for s in range(2):
                        po = s * 64
                        h = 2 * t + s
                        nc.tensor.matmul(
                            st["av"][po : po + VP, j * 512 : (j + 1) * 512],
                            lhsT=vT_sb[:, yt - 1, h, :],
                            rhs=p_half[s][yt - 1][j],
                            start=(yt == 1),
                            stop=(yt == 8),
                            skip_group_check=True,
                        )
            if yt == 8:
                # drain av to SBUF in halves (DVE + ACT), freeing the av
                # psum slot for the next pair
                avc = avcp.tile([P, SEQ], F32, tag="avc", name=f"avc{t}")
                nc.vector.tensor_copy(avc[:, 0:512], st["av"][:, 0:512])
                nc.scalar.activation(
                    avc[:, 512:SEQ], st["av"][:, 512:SEQ], IDENT, bias=zb_sb[:]
                )
                st["avc"] = avc

        # ---- the schedule: att blocks paced by the exp engines, with
        # qkv groups sprinkled into the PE's slack. Pair 0's q/k go
        # first; v groups early in pair 0 (av(t0,1) needs vT(0));
        # q1/k1 mid-pair-0, q2/k2 + q3/k3 across pair 1. ----
        for which in ("q", "k"):
            for j in range(2):
                qk_group(0, which, j)

        filler = {
            (0, 0): lambda: (v_group(0), v_group(1)),
            (0, 1): lambda: (v_group(2), v_group(3)),
            (0, 2): lambda: qk_group(1, "q", 0),
            (0, 3): lambda: qk_group(1, "q", 1),
            (0, 4): lambda: (v_group(4), v_group(5)),
            (0, 5): lambda: qk_group(1, "k", 0),
            (0, 6): lambda: (v_group(6), v_group(7)),
            (0, 7): lambda: qk_group(1, "k", 1),
            (1, 0): lambda: qk_group(2, "q", 0),
            (1, 1): lambda: qk_group(2, "q", 1),
            (1, 2): lambda: qk_group(2, "k", 0),
            (1, 3): lambda: qk_group(2, "k", 1),
            (1, 4): lambda: qk_group(3, "q", 0),
            (1, 5): lambda: qk_group(3, "q", 1),
            (1, 6): lambda: qk_group(3, "k", 0),
            (1, 7): lambda: qk_group(3, "k", 1),
        }
        # epilogue hooks: (pair being processed, yt) -> action on a
        # PREVIOUS pair. Pair 2's chain runs one yt earlier and
        # normalizes on DVE (its broadcast lands after exp is done).
        hooks = {
            (1, 1): lambda: epi_den(0),
            (1, 3): lambda: epi_recip(0),
            (1, 4): lambda: epi_bcast(0, "gpsimd"),
            (2, 1): lambda: epi_den(1),
            (2, 3): lambda: epi_recip(1),
            (2, 4): lambda: epi_bcast(1, "gpsimd"),
            (3, 0): lambda: epi_den(2),
            (3, 1): lambda: (epi_recip(2), epi_bcast(2, "gpsimd")),
        }

        for t in range(4):
            for yt in range(9):
                att_block(t, yt)
                f = filler.get((t, yt))
                if f:
                    f()
                h = hooks.get((t, yt))
                if h:
                    h()

        # ---- tail: fin(t3) + proj (kt-outer) ----
        # fin: scatter the denominator rows to 128 lanes (reciprocal is
        # ~6 cyc/element on DVE, so narrow-lane forms lose; DMA cannot
        # read PSUM so the scatters source the av_copy), reciprocal in
        # one 253ns op, scatter back to a row pair for the PE
        # ones-matmul broadcast.
        avc3 = epi[3]["avc"]
        den_pm3 = rpool.tile([P, 16], F32, tag="denpm")
        nc.sync.dma_start(den_pm3[:, 0:8], avc3[0:1, :])
        nc.scalar.dma_start(den_pm3[:, 8:16], avc3[64:65, :])
        rec_pm3 = rpool.tile([P, 16], BF16, tag="recpm")
        with nc.allow_low_precision(reason="softmax denom reciprocal to bf16"):
            nc.vector.reciprocal(rec_pm3[:], den_pm3[:])
        rec65 = rpool.tile([33, SEQ], BF16, tag="rec65")
        nc.sync.dma_start(rec65[0:1, :], rec_pm3[:, 0:8])
        nc.scalar.dma_start(rec65[32:33, :], rec_pm3[:, 8:16])

        # proj kt rounds 0-2 (kt0/kt1 start while pair 2/3 epilogues run)
        prh = [
            [ps_h.tile([P, 512], F32, tag="h", name=f"pr{ot}{j}") for j in range(2)]
            for ot in range(3)
        ]
        for kt in range(3):
            for ot in range(3):
                for j in range(2):
                    nc.tensor.matmul(
                        prh[ot][j][:],
                        lhsT=wp_sb[:, kt, ot * 128 : (ot + 1) * 128],
                        rhs=u_sb[kt][:, j * 512 : (j + 1) * 512],
                        start=(kt == 0),
                        stop=False,
                        skip_group_check=True,
                    )

        # final-pair broadcast: K=1 ones-matmuls into the freed av slot
        # (rows 0/64 of rec65 -> psum rows 0:64 / 64:128), then the DVE
        # multiply lands u3 just before kt3 needs it.
        bc_ps = ps_av.tile([P, SEQ], F32, tag="av", name="bc_ps")
        for s in range(2):
            sp = 32 * s
            for j in range(2):
                nc.tensor.matmul(
                    bc_ps[s * 64 : (s + 1) * 64, j * 512 : (j + 1) * 512],
                    lhsT=ones_sb[sp : sp + 1, :],
                    rhs=rec65[sp : sp + 1, j * 512 : (j + 1) * 512],
                    start=True,
                    stop=True,
                )
        nc.vector.tensor_tensor(u_sb[3][:], avc3[:], bc_ps[:], MULT)

        for ot in range(3):
            for j in range(2):
                nc.tensor.matmul(
                    prh[ot][j][:],
                    lhsT=wp_sb[:, 3, ot * 128 : (ot + 1) * 128],
                    rhs=u_sb[3][:, j * 512 : (j + 1) * 512],
                    start=False,
                    stop=True,
                    skip_group_check=True,
                )

        oq = [nc.sync, nc.gpsimd, nc.scalar]
        for ot in range(3):
            o_sb = opool.tile([P, SEQ], BF16, tag="o")
            # bias-copy halves spread over ACT/DVE so the three tiles
            # drain in parallel instead of serializing on ACT
            for j in range(2):
                src = prh[ot][j][:]
                dst = o_sb[:, j * 512 : (j + 1) * 512]
                if (ot + j) % 2 == 1:
                    nc.vector.tensor_scalar(
                        dst, src, bias_sb[:, ot : ot + 1], None, ADD
                    )
                else:
                    nc.scalar.activation(
                        dst, src, IDENT, bias=bias_sb[:, ot : ot + 1]
                    )
            oq[ot].dma_start(out_d.ap()[ot * 128 : (ot + 1) * 128, :], o_sb[:])


def build_nc():
    nc = bacc.Bacc("TRN2", target_bir_lowering=False, debug=False, num_devices=NCORES)
    x_d = nc.dram_tensor("x", [P, 3, SEQ], BF16, kind="ExternalInput")
    wq0_d = nc.dram_tensor("wq0", [P, 512], BF16, kind="ExternalInput")
    wq12_d = nc.dram_tensor("wq12", [P, 2, 512], BF16, kind="ExternalInput")
    wk0_d = nc.dram_tensor("wk0", [P, 512], BF16, kind="ExternalInput")
    wk12_d = nc.dram_tensor("wk12", [P, 2, 512], BF16, kind="ExternalInput")
    wv_d = nc.dram_tensor("wv", [P, 3, HEADS * VP], BF16, kind="ExternalInput")
    wp_d = nc.dram_tensor("wp", [P, 4, DIM], BF16, kind="ExternalInput")
    b_d = nc.dram_tensor("bias", [P, 3], F32, kind="ExternalInput")
    brow_d = nc.dram_tensor("brow", [2, 4, SEQ], BF16, kind="ExternalInput")
    out_d = nc.dram_tensor("out", [DIM, SEQ], BF16, kind="ExternalOutput")

    with tile.TileContext(nc) as tc:
        _emit(tc, nc, x_d, wq0_d, wq12_d, wk0_d, wk12_d, wv_d, wp_d, b_d, brow_d, out_d)
    nc.compile()
    return nc


def pack_inputs(x, w_qkv, w_proj, b_proj):
    """Host-side weight packing. Returns per-core input maps."""
    import ml_dtypes

    x = np.asarray(x, np.float32)
    w_qkv = np.asarray(w_qkv, np.float32)
    w_proj = np.asarray(w_proj, np.float32)
    b_proj = np.asarray(b_proj, np.float32)
    scale = DH ** -0.5
    w_q, w_k, w_v = w_qkv[0:DIM], w_qkv[DIM : 2 * DIM], w_qkv[2 * DIM :]

    WQ = np.zeros((DIM, 512), np.float32)
    WK = np.zeros((DIM, 512), np.float32)
    WV = np.zeros((DIM, HEADS * VP), np.float32)
    WP = np.zeros((512, DIM), np.float32)
    for h in range(HEADS):
        col = (h // 2) * 128 + (h % 2) * 64
        # odd heads run the DVE Schraudolph path: fold A into the scale
        qs = scale * (A_EXP if h % 2 == 1 else 1.0)
        WQ[:, col : col + DH] = (w_q[h * DH : (h + 1) * DH] * qs).T
        WK[:, col : col + DH] = w_k[h * DH : (h + 1) * DH].T
        WV[:, h * VP + 16 : h * VP + 16 + DH] = w_v[h * DH : (h + 1) * DH].T
        WP[col + 16 : col + 16 + DH, :] = w_proj[:, h * DH : (h + 1) * DH].T
    BIAS = np.ascontiguousarray(b_proj.reshape(3, P).T)

    def pm(a, chunks):
        # [(chunks*P), f] -> [P, chunks, f] partition-major bf16 pre-layout
        return np.ascontiguousarray(
            a.reshape(chunks, P, a.shape[-1]).transpose(1, 0, 2)
        ).astype(ml_dtypes.bfloat16)

    WQp, WKp, WVp, WPp = pm(WQ, 3), pm(WK, 3), pm(WV, 3), pm(WP, 4)
    WQ0 = np.ascontiguousarray(WQp[:, 0, :])
    WQ12 = np.ascontiguousarray(WQp[:, 1:3, :])
    WK0 = np.ascontiguousarray(WKp[:, 0, :])
    WK12 = np.ascontiguousarray(WKp[:, 1:3, :])
    BROW = np.empty((2, 4, SEQ), np.float32)
    BROW[0] = 1.0
    BROW[1] = B_EXP
    BROW = BROW.astype(ml_dtypes.bfloat16)
    in_maps = []
    for b in range(NCORES):
        in_maps.append(
            {
                "x": pm(x[b].reshape(DIM, SEQ), 3),
                "wq0": WQ0,
                "wq12": WQ12,
                "wk0": WK0,
                "wk12": WK12,
                "wv": WVp,
                "wp": WPp,
                "bias": BIAS,
                "brow": BROW,
            }
        )
    return in_maps


def run(in_maps, trace=False):
    if "nc" not in _NC_CACHE:
        _NC_CACHE["nc"] = build_nc()
    nc = _NC_CACHE["nc"]
    res = run_bass_kernel_spmd(
        nc, in_maps, core_ids=list(range(NCORES)), trace=trace
    )
    out = np.stack(
        [res.results[i]["out"].astype(np.float32) for i in range(NCORES)]
    )
    return out.reshape(NCORES, DIM, 32, 32), res


def kernel(x, w_qkv, w_proj, b_proj):
    out, _ = run(pack_inputs(x, w_qkv, w_proj, b_proj))
    return out


# revision 22
# speedup vs baseline: 1.2399x; 1.0224x over previous
"""Trainium2 Bass kernel for nn_Attention_57166014709861.

8-batch image attention (B=8, C=384, h=8, d=48, HW=1024), data-parallel:
one batch image per NeuronCore, weights broadcast, host-side gather.

v3 (from v2 107us, v1 119-129us): the whole kernel is ONE software
pipeline paced by the exp engines (ACT+DVE), which own the critical
path (~74us of psum->sbuf work split across two engines).

  * Skewed emission: attention blocks att(t,yt) start right after pair
    0's q/k land (~15us instead of ~33us); the remaining qkv groups
    (q1-3/k1-3, v) are sprinkled one-per-yt into the attention stream so
    the PE (which has ~50% slack at the exp-bound pace) computes them in
    the gaps and HAM stays warm.
  * PSUM in single-bank [128,512] halves (6 rotating "h" slots shared by
    qkv/v/sT/proj): paired matmuls run CONCURRENTLY on disjoint PE
    sub-arrays (row groups 0/64 for the K<=49 sT, col groups 0/64 for
    the M=64 av) -- tile_position auto-derives from base partitions.
  * Static per-head exp split: even heads (s=0, partitions 0:48) use ACT
    native Exp; odd heads (s=1, partitions 64:113) use DVE. For DVE
    heads the Schraudolph affine map x = A*s + B is folded INTO the sT
    matmul (A rides the host-packed w_q scale, B rides contraction row
    112: q row 112 = 1, k row 112 = B, delivered by a tiny dep-free DMA
    since engines cannot address base partition 112; the q/k copies only
    write partitions 0:112 so the rows survive). DVE then runs a pure
    copy-convert f32->u16: trunc(x) viewed as bf16 bits == bf16(exp(s)),
    zero-mean +-4% err. bf16 rounds B to 16256 -- an integer shift of B
    is an exact constant factor on p that cancels in the normalization.
    Softmax-argument noise does NOT average away, so B must stay
    zero-mean-calibrated and anything coarser than bf16 on the q/k path
    blows the 2e-2 gate (fp8 measured 4.9e-2).
  * Epilogue re-phased to avoid engine FIFO head-of-line blocking: the
    denominator-scatter DMAs fire at the next pair's yt1/yt0, the DVE
    reciprocal only at yt3/yt2 (its dep already landed -> no stall), the
    broadcast DMA at yt4/yt3. Pairs 0/1 normalize on GPSIMD; pair 2 on
    DVE (its broadcast lands after exp is done, and GPSIMD's queue would
    be blocked); pair 3 skips the DMA bounce entirely: 65-lane DVE
    reciprocal straight off av_copy rows 0..64, K=1 ones-matmul
    broadcast on the PE, DVE multiply from PSUM.
  * proj is kt-outer (all three output tiles' kt round together) so each
    round starts the moment its u tile lands; kt3 lands last after the
    final pair's u3.
"""

import sys

if "/opt/trn_rl_repo" not in sys.path:
    sys.path.insert(0, "/opt/trn_rl_repo")

import numpy as np

import concourse.bass as bass
import concourse.mybir as mybir
import concourse.tile as tile
from concourse import bacc
from concourse.bass_utils import run_bass_kernel_spmd

DIM = 384
HEADS = 8
DH = 48
SEQ = 1024
P = 128
NCORES = 8
VP = 64  # packed v cols per head: ones at col 0, zeros 1-15, 48 data at 16-63

F32 = mybir.dt.float32
BF16 = mybir.dt.bfloat16
U16 = mybir.dt.uint16
EXP = mybir.ActivationFunctionType.Exp
IDENT = mybir.ActivationFunctionType.Identity
ADD = mybir.AluOpType.add
MULT = mybir.AluOpType.mult

# Schraudolph exp-as-bf16-bits: trunc(s*A + B) viewed as bf16 ~= exp(s).
# A = 128/ln2; B calibrated for ZERO-MEAN rel err (max 4.2%, rms 1.8%)
# under truncation.
A_EXP = 184.6649652337873
B_EXP = 16249.125

_NC_CACHE = {}


def _emit(tc, nc, x_d, wq0_d, wq12_d, wk0_d, wk12_d, wv_d, wp_d, b_d, brow_d, out_d):
    with (
        tc.tile_pool(name="const", bufs=1) as constp,
        tc.tile_pool(name="weights", bufs=1) as wpool,
        tc.tile_pool(name="data", bufs=1) as data,
        tc.tile_pool(name="ptile", bufs=16) as ppool,
        tc.tile_pool(name="bcpool", bufs=3) as bcpool,
        tc.tile_pool(name="rpool", bufs=3) as rpool,
        tc.tile_pool(name="avcp", bufs=3) as avcp,
        tc.tile_pool(name="opool", bufs=3) as opool,
        tc.tile_pool(name="ps_h", bufs=6, space="PSUM") as ps_h,
        tc.tile_pool(name="ps_av", bufs=1, space="PSUM") as ps_av,
        tc.tile_pool(name="dram", bufs=3, space="DRAM") as drampool,
    ):
        # ---- loads (bf16, 3 queues). Dependency tracking is per-tile
        # (with subtile refinement): x per ko chunk, wq/wk split ko0 vs
        # ko12 so pair 0's q/k matmuls only wait for x+wq0/wk0. x2 rides
        # the scalar queue so all of x lands by ~13us (q0/k0 accumulate
        # all three ko chunks before the first sT can go).
        x_ch = [
            data.tile([P, SEQ], BF16, tag=f"x{ko}", name=f"x{ko}")
            for ko in range(3)
        ]
        wq0_sb = wpool.tile([P, 512], BF16, tag="wq0")
        wq12_sb = wpool.tile([P, 2, 512], BF16, tag="wq12")
        wk0_sb = wpool.tile([P, 512], BF16, tag="wk0")
        wk12_sb = wpool.tile([P, 2, 512], BF16, tag="wk12")
        wv_sb = wpool.tile([P, 3, HEADS * VP], BF16, tag="wv")
        wp_sb = wpool.tile([P, 4, DIM], BF16, tag="wp")
        bias_sb = constp.tile([P, 3], F32, tag="bias")

        q_sb = data.tile([P, 4, SEQ], BF16, tag="q")
        k_sb = data.tile([P, 4, SEQ], BF16, tag="k")
        vT_sb = data.tile([P, 8, HEADS, VP], BF16, tag="vT")

        nc.sync.dma_start(x_ch[0][:], x_d.ap()[:, 0, :])
        nc.scalar.dma_start(wq0_sb[:], wq0_d.ap())
        nc.gpsimd.dma_start(wk0_sb[:], wk0_d.ap())
        # Schraudolph affine rows (dep-free: q/k copies never write
        # partition 112+): q row 112 = 1, k row 112 = B.
        nc.gpsimd.dma_start(q_sb[112:113, :, :], brow_d.ap()[0:1, :, :])
        nc.gpsimd.dma_start(k_sb[112:113, :, :], brow_d.ap()[1:2, :, :])
        nc.sync.dma_start(x_ch[1][:], x_d.ap()[:, 1, :])
        nc.scalar.dma_start(x_ch[2][:], x_d.ap()[:, 2, :])
        nc.gpsimd.dma_start(wk12_sb[:], wk12_d.ap())
        nc.scalar.dma_start(wq12_sb[:], wq12_d.ap())
        nc.gpsimd.dma_start(wv_sb[:], wv_d.ap())
        nc.gpsimd.dma_start(wp_sb[:], wp_d.ap())
        nc.scalar.dma_start(bias_sb[:], b_d.ap())

        def wq_ko(ko):
            return wq0_sb[:] if ko == 0 else wq12_sb[:, ko - 1, :]

        def wk_ko(ko):
            return wk0_sb[:] if ko == 0 else wk12_sb[:, ko - 1, :]

        zb_sb = constp.tile([P, 1], F32, tag="zb")
        nc.gpsimd.memset(zb_sb[:], 0.0)
        # rows 0 and 64 are used as K=1 matmul lhsT for the final-pair
        # broadcast, so the ones tile spans 65 partitions.
        ones_sb = constp.tile([65, 64], BF16, tag="ones")
        nc.gpsimd.memset(ones_sb[:], 1.0)

        # ---- copy engine rotation (GPSIMD cannot read PSUM) ----
        _ncopy = [0]

        def eng_copy(dst, src):
            if _ncopy[0] % 2 == 0:
                nc.vector.tensor_copy(dst, src)
            else:
                nc.scalar.activation(
                    dst, src, IDENT, bias=zb_sb[0 : src.shape[0], :]
                )
            _ncopy[0] += 1

        # ---- qkv work groups (emitted interleaved into the attention
        # stream below). q/k copies write partitions 0:112 only, so the
        # Schraudolph rows survive. ----
        def qk_group(t, which, j):
            dst, wf = (q_sb, wq_ko) if which == "q" else (k_sb, wk_ko)
            ps = ps_h.tile([P, 512], F32, tag="h", name="qk_ps")
            for ko in range(3):
                nc.tensor.matmul(
                    ps[:],
                    lhsT=wf(ko)[:, t * 128 : (t + 1) * 128],
                    rhs=x_ch[ko][:, j * 512 : (j + 1) * 512],
                    start=(ko == 0),
                    stop=(ko == 2),
                )
            eng_copy(dst[0:112, t, j * 512 : (j + 1) * 512], ps[0:112, :])

        def v_group(yt):
            ps = ps_h.tile([P, 512], F32, tag="h", name="v_ps")
            for ko in range(3):
                nc.tensor.matmul(
                    ps[:],
                    lhsT=x_ch[ko][:, yt * 128 : (yt + 1) * 128],
                    rhs=wv_sb[:, ko, :],
                    start=(ko == 0),
                    stop=(ko == 2),
                )
            eng_copy(
                vT_sb[:, yt, :, :],
                ps[:].rearrange("p (h v) -> p h v", h=HEADS),
            )
            # ones column (col 0) for the softmax denominator (psum row 0/64)
            nc.gpsimd.memset(vT_sb[:, yt, :, 0:1], 1.0)

        # ---- attention state ----
        u_sb = [data.tile([P, SEQ], BF16, tag=f"u{i}", name=f"u{i}") for i in range(4)]
        epi = {}  # pair -> dict with av_copy / den_dram / rec_dram

        # Epilogue for pairs 0-2, phase-split so no engine FIFO-blocks on
        # a DMA that hasn't landed: den scatter DMAs early, reciprocal
        # two yts later (dep already met), broadcast next.
        def epi_den(t):
            st = epi[t]
            den_dram = drampool.tile([2 * SEQ], F32, tag="den")
            nc.sync.dma_start(den_dram[0:SEQ], st["avc"][0:1, :])
            nc.sync.dma_start(den_dram[SEQ : 2 * SEQ], st["avc"][64:65, :])
            den_pm = rpool.tile([P, 16], F32, tag="denpm")
            nc.sync.dma_start(den_pm[:], den_dram[:].rearrange("(p f) -> p f", p=P))
            st["den_pm"] = den_pm

        def epi_recip(t):
            st = epi[t]
            rec_pm = rpool.tile([P, 16], BF16, tag="recpm")
            with nc.allow_low_precision(reason="softmax denom reciprocal to bf16"):
                nc.vector.reciprocal(rec_pm[:], st["den_pm"][:])
            rec_dram = drampool.tile([2 * SEQ], BF16, tag="rec")
            nc.sync.dma_start(rec_dram[:], rec_pm[:])
            st["rec_dram"] = rec_dram

        def epi_bcast(t, split=False):
            st = epi[t]
            bc_sb = bcpool.tile([P, SEQ], BF16, tag="bcs")
            rec_r = st["rec_dram"][:].rearrange("(h f) -> h f", h=2)
            if split:
                # the stride-0 expansion runs at ~25GB/s per queue
                # (128 tiny descriptors); split across two queues when
                # the landing time matters (pair 2 gates proj kt2)
                nc.sync.dma_start(
                    bc_sb[0:64, :], rec_r[0:1, None, :].to_broadcast([1, 64, SEQ])
                )
                nc.scalar.dma_start(
                    bc_sb[64:128, :], rec_r[1:2, None, :].to_broadcast([1, 64, SEQ])
                )
            else:
                nc.sync.dma_start(
                    bc_sb[:], rec_r[:, None, :].to_broadcast([2, 64, SEQ])
                )
            # all-SBUF multiply on Pool keeps DVE free for exp
            nc.gpsimd.tensor_tensor(u_sb[t][:], st["avc"][:], bc_sb[:], MULT)

        # ---- one attention block: sT(yt) + exp, av(yt-1) ----
        def att_block(t, yt):
            st = epi.setdefault(t, {"p": [[[None] * 2 for _ in range(8)] for _ in range(2)]})
            p_half = st["p"]
            if yt < 8:
                for j in range(2):
                    for s in range(2):
                        po, ke = (0, 48) if s == 0 else (64, 49)
                        hp = ps_h.tile([P, 512], F32, tag="h", name="sT_h")
                        nc.tensor.matmul(
                            hp[:],
                            lhsT=k_sb[po : po + ke, t, yt * 128 : (yt + 1) * 128],
                            rhs=q_sb[po : po + ke, t, j * 512 : (j + 1) * 512],
                            start=True,
                            stop=True,
                        )
                        if s == 0:
                            pt = ppool.tile([P, 512], BF16, tag="p", name="p_a")
                            nc.scalar.activation(pt[:], hp[:], EXP, bias=zb_sb[:])
                            p_half[s][yt][j] = pt[:]
                        else:
                            pt = ppool.tile([P, 512], U16, tag="p", name="p_d")
                            nc.vector.tensor_copy(pt[:], hp[:])
                            p_half[s][yt][j] = pt[:].bitcast(BF16)
            if yt > 0:
                if yt == 1:
                    st["av"] = ps_av.tile([P, SEQ], F32, tag="av", name=f"av{t}")
                for j in range(2):
                    for s in range(2):
                        po = s * 64
                        h = 2 * t + s
                        nc.tensor.matmul(
                            st["av"][po : po + VP, j * 512 : (j + 1) * 512],
                            lhsT=vT_sb[:, yt - 1, h, :],
                            rhs=p_half[s][yt - 1][j],
                            start=(yt == 1),
                            stop=(yt == 8),
                            skip_group_check=True,
                        )
            if yt == 8:
                # drain av to SBUF in halves (DVE + ACT), freeing the av
                # psum slot for the next pair
                avc = avcp.tile([P, SEQ], F32, tag="avc", name=f"avc{t}")
                nc.vector.tensor_copy(avc[:, 0:512], st["av"][:, 0:512])
                nc.scalar.activation(
                    avc[:, 512:SEQ], st["av"][:, 512:SEQ], IDENT, bias=zb_sb[:]
                )
                st["avc"] = avc

        # ---- PE warmup: HAM gates the PE clock to 1.2 GHz until it has
        # seen ~3.4us of sustained activity. Burn the input-DMA wait
        # (~7.1-11us) on tiny K=1 matmuls so q0/k0 run at 2.4 GHz. ----
        def pe_warmup(n):
            wm = ps_h.tile([P, 512], F32, tag="h", name="warm_ps")
            for _ in range(n):
                nc.tensor.matmul(
                    wm[0:64, 0:64],
                    lhsT=ones_sb[0:1, :],
                    rhs=ones_sb[0:1, :],
                    start=True,
                    stop=True,
                )

        pe_warmup(36)

        # ---- the schedule: att blocks paced by the exp engines, with
        # qkv groups sprinkled into the PE's slack. Pair 0's q/k go
        # first; v groups early in pair 0 (av(t0,1) needs vT(0));
        # q1/k1 mid-pair-0, q2/k2 + q3/k3 across pair 1. ----
        for which in ("q", "k"):
            for j in range(2):
                qk_group(0, which, j)

        filler = {
            (0, 0): lambda: (v_group(0), v_group(1)),
            (0, 1): lambda: (v_group(2), v_group(3)),
            (0, 2): lambda: qk_group(1, "q", 0),
            (0, 3): lambda: qk_group(1, "q", 1),
            (0, 4): lambda: (v_group(4), v_group(5)),
            (0, 5): lambda: qk_group(1, "k", 0),
            (0, 6): lambda: (v_group(6), v_group(7)),
            (0, 7): lambda: qk_group(1, "k", 1),
            (1, 0): lambda: qk_group(2, "q", 0),
            (1, 1): lambda: qk_group(2, "q", 1),
            (1, 2): lambda: qk_group(2, "k", 0),
            (1, 3): lambda: qk_group(2, "k", 1),
            (1, 4): lambda: qk_group(3, "q", 0),
            (1, 5): lambda: qk_group(3, "q", 1),
            (1, 6): lambda: qk_group(3, "k", 0),
            (1, 7): lambda: qk_group(3, "k", 1),
        }
        # epilogue hooks: (pair being processed, yt) -> action on a
        # PREVIOUS pair. Pair 2's chain runs one yt earlier and
        # normalizes on DVE (its broadcast lands after exp is done).
        hooks = {
            (1, 1): lambda: epi_den(0),
            (1, 3): lambda: epi_recip(0),
            (1, 4): lambda: epi_bcast(0),
            (2, 1): lambda: epi_den(1),
            (2, 3): lambda: epi_recip(1),
            (2, 4): lambda: epi_bcast(1),
            (3, 0): lambda: epi_den(2),
            (3, 1): lambda: (epi_recip(2), epi_bcast(2, split=True)),
        }

        for t in range(4):
            for yt in range(9):
                att_block(t, yt)
                f = filler.get((t, yt))
                if f:
                    f()
                h = hooks.get((t, yt))
                if h:
                    h()

        # ---- tail: fin(t3) + proj (kt-outer) ----
        # fin: scatter the denominator rows to 128 lanes (reciprocal is
        # ~6 cyc/element on DVE, so narrow-lane forms lose; DMA cannot
        # read PSUM so the scatters source the av_copy), reciprocal in
        # one 253ns op, scatter back to a row pair for the PE
        # ones-matmul broadcast.
        avc3 = epi[3]["avc"]
        den_pm3 = rpool.tile([P, 16], F32, tag="denpm")
        nc.sync.dma_start(den_pm3[:, 0:8], avc3[0:1, :])
        nc.scalar.dma_start(den_pm3[:, 8:16], avc3[64:65, :])
        rec_pm3 = rpool.tile([P, 16], BF16, tag="recpm")
        with nc.allow_low_precision(reason="softmax denom reciprocal to bf16"):
            nc.vector.reciprocal(rec_pm3[:], den_pm3[:])
        rec65 = rpool.tile([33, SEQ], BF16, tag="rec65")
        nc.sync.dma_start(rec65[0:1, :], rec_pm3[:, 0:8])
        nc.scalar.dma_start(rec65[32:33, :], rec_pm3[:, 8:16])

        # proj kt rounds 0-2 (kt0/kt1 start while pair 2/3 epilogues run)
        prh = [
            [ps_h.tile([P, 512], F32, tag="h", name=f"pr{ot}{j}") for j in range(2)]
            for ot in range(3)
        ]
        for kt in range(2):
            for ot in range(3):
                for j in range(2):
                    nc.tensor.matmul(
                        prh[ot][j][:],
                        lhsT=wp_sb[:, kt, ot * 128 : (ot + 1) * 128],
                        rhs=u_sb[kt][:, j * 512 : (j + 1) * 512],
                        start=(kt == 0),
                        stop=False,
                        skip_group_check=True,
                    )

        # keep HAM warm across the u2/u3 wait (PE would otherwise idle
        # >3.4us here and the remaining matmuls would run at 1.2 GHz)
        pe_warmup(24)

        # final-pair broadcast: K=1 ones-matmuls into the freed av slot
        # (rows 0/64 of rec65 -> psum rows 0:64 / 64:128) BEFORE the kt2
        # round (kt2 waits on pair 2's broadcast; bc must not queue
        # behind it), then the DVE multiply lands u3 before kt3.
        bc_ps = ps_av.tile([P, SEQ], F32, tag="av", name="bc_ps")
        for s in range(2):
            sp = 32 * s
            for j in range(2):
                nc.tensor.matmul(
                    bc_ps[s * 64 : (s + 1) * 64, j * 512 : (j + 1) * 512],
                    lhsT=ones_sb[sp : sp + 1, :],
                    rhs=rec65[sp : sp + 1, j * 512 : (j + 1) * 512],
                    start=True,
                    stop=True,
                )
        nc.vector.tensor_tensor(u_sb[3][:], avc3[:], bc_ps[:], MULT)

        for ot in range(3):
            for j in range(2):
                nc.tensor.matmul(
                    prh[ot][j][:],
                    lhsT=wp_sb[:, 2, ot * 128 : (ot + 1) * 128],
                    rhs=u_sb[2][:, j * 512 : (j + 1) * 512],
                    start=False,
                    stop=False,
                    skip_group_check=True,
                )

        for ot in range(3):
            for j in range(2):
                nc.tensor.matmul(
                    prh[ot][j][:],
                    lhsT=wp_sb[:, 3, ot * 128 : (ot + 1) * 128],
                    rhs=u_sb[3][:, j * 512 : (j + 1) * 512],
                    start=False,
                    stop=True,
                    skip_group_check=True,
                )

        oq = [nc.sync, nc.gpsimd, nc.scalar]
        for ot in range(3):
            o_sb = opool.tile([P, SEQ], BF16, tag="o")
            # bias-copy halves spread over ACT/DVE so the three tiles
            # drain in parallel instead of serializing on ACT
            for j in range(2):
                src = prh[ot][j][:]
                dst = o_sb[:, j * 512 : (j + 1) * 512]
                if (ot + j) % 2 == 1:
                    nc.vector.tensor_scalar(
                        dst, src, bias_sb[:, ot : ot + 1], None, ADD
                    )
                else:
                    nc.scalar.activation(
                        dst, src, IDENT, bias=bias_sb[:, ot : ot + 1]
                    )
            oq[ot].dma_start(out_d.ap()[ot * 128 : (ot + 1) * 128, :], o_sb[:])


def build_nc():
    nc = bacc.Bacc("TRN2", target_bir_lowering=False, debug=False, num_devices=NCORES)
    x_d = nc.dram_tensor("x", [P, 3, SEQ], BF16, kind="ExternalInput")
    wq0_d = nc.dram_tensor("wq0", [P, 512], BF16, kind="ExternalInput")
    wq12_d = nc.dram_tensor("wq12", [P, 2, 512], BF16, kind="ExternalInput")
    wk0_d = nc.dram_tensor("wk0", [P, 512], BF16, kind="ExternalInput")
    wk12_d = nc.dram_tensor("wk12", [P, 2, 512], BF16, kind="ExternalInput")
    wv_d = nc.dram_tensor("wv", [P, 3, HEADS * VP], BF16, kind="ExternalInput")
    wp_d = nc.dram_tensor("wp", [P, 4, DIM], BF16, kind="ExternalInput")
    b_d = nc.dram_tensor("bias", [P, 3], F32, kind="ExternalInput")
    brow_d = nc.dram_tensor("brow", [2, 4, SEQ], BF16, kind="ExternalInput")
    out_d = nc.dram_tensor("out", [DIM, SEQ], BF16, kind="ExternalOutput")

    with tile.TileContext(nc) as tc:
        _emit(tc, nc, x_d, wq0_d, wq12_d, wk0_d, wk12_d, wv_d, wp_d, b_d, brow_d, out_d)
    nc.compile()
    return nc


def pack_inputs(x, w_qkv, w_proj, b_proj):
    """Host-side weight packing. Returns per-core input maps."""
    import ml_dtypes

    x = np.asarray(x, np.float32)
    w_qkv = np.asarray(w_qkv, np.float32)
    w_proj = np.asarray(w_proj, np.float32)
    b_proj = np.asarray(b_proj, np.float32)
    scale = DH ** -0.5
    w_q, w_k, w_v = w_qkv[0:DIM], w_qkv[DIM : 2 * DIM], w_qkv[2 * DIM :]

    WQ = np.zeros((DIM, 512), np.float32)
    WK = np.zeros((DIM, 512), np.float32)
    WV = np.zeros((DIM, HEADS * VP), np.float32)
    WP = np.zeros((512, DIM), np.float32)
    for h in range(HEADS):
        col = (h // 2) * 128 + (h % 2) * 64
        # odd heads run the DVE Schraudolph path: fold A into the scale
        qs = scale * (A_EXP if h % 2 == 1 else 1.0)
        WQ[:, col : col + DH] = (w_q[h * DH : (h + 1) * DH] * qs).T
        WK[:, col : col + DH] = w_k[h * DH : (h + 1) * DH].T
        WV[:, h * VP + 16 : h * VP + 16 + DH] = w_v[h * DH : (h + 1) * DH].T
        WP[col + 16 : col + 16 + DH, :] = w_proj[:, h * DH : (h + 1) * DH].T
    BIAS = np.ascontiguousarray(b_proj.reshape(3, P).T)

    def pm(a, chunks):
        # [(chunks*P), f] -> [P, chunks, f] partition-major bf16 pre-layout
        return np.ascontiguousarray(
            a.reshape(chunks, P, a.shape[-1]).transpose(1, 0, 2)
        ).astype(ml_dtypes.bfloat16)

    WQp, WKp, WVp, WPp = pm(WQ, 3), pm(WK, 3), pm(WV, 3), pm(WP, 4)
    WQ0 = np.ascontiguousarray(WQp[:, 0, :])
    WQ12 = np.ascontiguousarray(WQp[:, 1:3, :])
    WK0 = np.ascontiguousarray(WKp[:, 0, :])
    WK12 = np.ascontiguousarray(WKp[:, 1:3, :])
    BROW = np.empty((2, 4, SEQ), np.float32)
    BROW[0] = 1.0
    BROW[1] = B_EXP
    BROW = BROW.astype(ml_dtypes.bfloat16)
    in_maps = []
    for b in range(NCORES):
        in_maps.append(
            {
                "x": pm(x[b].reshape(DIM, SEQ), 3),
                "wq0": WQ0,
                "wq12": WQ12,
                "wk0": WK0,
                "wk12": WK12,
                "wv": WVp,
                "wp": WPp,
                "bias": BIAS,
                "brow": BROW,
            }
        )
    return in_maps


def run(in_maps, trace=False):
    if "nc" not in _NC_CACHE:
        _NC_CACHE["nc"] = build_nc()
    nc = _NC_CACHE["nc"]
    res = run_bass_kernel_spmd(
        nc, in_maps, core_ids=list(range(NCORES)), trace=trace
    )
    out = np.stack(
        [res.results[i]["out"].astype(np.float32) for i in range(NCORES)]
    )
    return out.reshape(NCORES, DIM, 32, 32), res


def kernel(x, w_qkv, w_proj, b_proj):
    out, _ = run(pack_inputs(x, w_qkv, w_proj, b_proj))
    return out
